# revision 1
# baseline (speedup 1.0000x reference)
"""Cross-attention kernel for Trainium2, distributed over 8 NeuronCores.

Sharding: data-parallel over batch (4) x tensor-parallel over head groups (2).
Core c handles batch b = c//2, heads [4g, 4g+4) with g = c%2.

Per-core device pipeline (layouts chosen so no on-device transposes are
needed; x^T / context^T are produced host-side as part of sharding):
  qT  = tanh(Wq_g^T @ x^T) * qmask          [256, 2048]   (d on partitions)
  kT  = tanh(Wk_g^T @ ctx^T), null col, pad [256, 2176]
  v   = ctx @ Wv_g (+ null row, ones col)   [2176, 4x65]  (j on partitions)
  S^T = exp(0.125 * kT_h^T qT_h + cmbias)   per (head, jtile, ichunk)
  outT_h = v_aug^T @ S^T  (row 64 = softmax denominator)
  rank-1 correction for masked queries, divide by denominator,
  out_partial = O @ Wo_g                    [2048, 512]
Host sums the two head-group partials per batch and adds bo.

PE instructions on TRN2 can carry at most ONE sync wait (walrus S3_LW /
ENGINE_NOP structs); Tile sometimes assigns more. `_split_pe_waits` runs
after scheduling and hoists extra waits onto PE nops inserted immediately
before the offending instruction — same engine stream, same blocking
semantics.
"""

import numpy as np

import concourse.bass as bass
import concourse.tile as tile
from concourse import bacc, bass_utils, mybir

FP = mybir.dt.float32
AF = mybir.ActivationFunctionType

B, N, M, DIM = 4, 2048, 2048, 512
HEADS, DH = 8, 64
G = 2          # head groups (tensor-parallel degree)
HG = 4         # heads per group
DG = HG * DH   # 256 dims per group
JT = 17        # j tiles of 128: 2048 context + null + 127 pad
JP = JT * 128  # 2176
NEG = -50.0    # additive mask bias (exp(-50) ~ 2e-22)
SCALE = 1.0 / np.sqrt(DH)  # 0.125
IC = 4         # i chunks of 512
VW = DH + 1    # v columns per head incl. ones column (den row)

LAST_RESULTS = None
_CACHE = {}


def _build():
    nc = bacc.Bacc("TRN2", debug=False, num_devices=8, enable_partition_id=False)
    d = {}

    def inp(name, shape):
        d[name] = nc.dram_tensor(name, shape, FP, kind="ExternalInput").ap()

    inp("xT", [DIM, N])
    inp("cxT", [DIM, M])
    inp("wq", [DIM, DG])
    inp("wk", [DIM, DG])
    inp("wv", [DIM, DG])
    inp("wo", [DG, DIM])
    inp("qm", [1, N])         # query mask as f32 row
    inp("cmf", [128, JT])     # context mask, padded+null, partition-major
    inp("nk", [128, 1])       # null_key tiled x2
    inp("nv", [1, HG * DH])   # null_value tiled x4
    d["out"] = nc.dram_tensor("out", [N, DIM], FP, kind="ExternalOutput").ap()

    with tile.TileContext(nc) as tc:
        _body(tc, d)
    nc.compile()
    return nc


_SPLIT_SKIP = (
    "InstDrain", "InstUnconditionalBranch", "InstCall",
    "InstEventSemaphore", "InstRegisterMove", "InstDmaTrigger",
)


def _split_pe_waits(nc):
    """Hoist all-but-one sync waits from compute-engine instructions onto
    fresh same-engine nops placed immediately before them (TRN2 TPB
    instruction structs accept only one sync wait in walrus codegen;
    drains/branches/DMA handle waits differently)."""
    engines = {
        mybir.EngineType.PE: nc.tensor,
        mybir.EngineType.Activation: nc.scalar,
        mybir.EngineType.DVE: nc.vector,
        mybir.EngineType.Pool: nc.gpsimd,
        mybir.EngineType.SP: nc.sync,
    }
    total = 0
    for bb in nc.m.functions[0].blocks:
        new_insts = []
        for ins in bb.instructions:
            si = ins.sync_info
            eng = engines.get(getattr(ins, "engine", None))
            if (
                eng is not None
                and type(ins).__name__ not in _SPLIT_SKIP
                and si is not None
                and si.on_wait
                and len(si.on_wait) > 1
            ):
                waits = list(si.on_wait)
                for w in waits[:-1]:
                    nop = eng._isa(
                        nc.isa.Opcode.NEURON_ISA_TPB_OPCODE_ENGINE_NOP,
                        {}, None, [], [], True,
                    )
                    nop.sync_info = mybir.SyncInfo(on_wait=[w], on_update=[])
                    nc.inst_map[nop.name] = nop
                    new_insts.append(nop)
                    total += 1
                si.on_wait = waits[-1:]
            new_insts.append(ins)
        bb.instructions = new_insts
    return total


def _body(tc, d):
    nc = tc.nc

    with (
        tc.tile_pool(name="consts", bufs=1) as consts,
        tc.tile_pool(name="big", bufs=1) as big,
        tc.tile_pool(name="spool", bufs=3) as spool,
        tc.tile_pool(name="small", bufs=2) as small,
        tc.tile_pool(name="mm", bufs=2, space="PSUM") as mm_ps,
        tc.tile_pool(name="acc", bufs=4, space="PSUM") as acc_ps,
        tc.tile_pool(name="rps", bufs=1, space="PSUM") as r_ps,
    ):
        # ---- constants / inputs ----
        wq = consts.tile([128, 4, DG], FP)
        nc.sync.dma_start(wq[:], d["wq"].rearrange("(c p) d -> p c d", p=128))
        wk = consts.tile([128, 4, DG], FP)
        nc.sync.dma_start(wk[:], d["wk"].rearrange("(c p) d -> p c d", p=128))
        wv = consts.tile([128, 4, DG], FP)
        nc.sync.dma_start(wv[:], d["wv"].rearrange("(c p) d -> p c d", p=128))
        wo = consts.tile([128, 2, DIM], FP)
        nc.sync.dma_start(wo[:], d["wo"].rearrange("(c p) o -> p c o", p=128))

        xT = big.tile([128, 4, N], FP)
        nc.sync.dma_start(xT[:], d["xT"].rearrange("(c p) i -> p c i", p=128))
        cxT = big.tile([128, 4, M], FP)
        nc.sync.dma_start(cxT[:], d["cxT"].rearrange("(c p) j -> p c j", p=128))

        qmB = big.tile([128, N], FP)  # query mask broadcast to 128 partitions
        nc.sync.dma_start(qmB[:], d["qm"].to_broadcast((128, N)))
        one_minus_qm = consts.tile([1, N], FP)
        nc.sync.dma_start(one_minus_qm[:], d["qm"])
        nc.scalar.activation(one_minus_qm[:], one_minus_qm[:], AF.Identity,
                             scale=-1.0, bias=1.0)

        cmf = consts.tile([128, JT], FP)
        nc.sync.dma_start(cmf[:], d["cmf"])
        negb = consts.tile([128, 1], FP)
        nc.vector.memset(negb[:], NEG)
        cmb = consts.tile([128, JT], FP)   # 0 where attendable, NEG where masked
        nc.scalar.activation(cmb[:], cmf[:], AF.Identity, scale=-NEG, bias=negb[:])
        cmexp = consts.tile([128, JT], FP)  # exp(cmb)
        nc.scalar.activation(cmexp[:], cmb[:], AF.Exp)
        negcm = consts.tile([128, JT], FP)  # -exp(cmb)
        nc.scalar.activation(negcm[:], cmexp[:], AF.Copy, scale=-1.0)

        nk = consts.tile([128, 1], FP)
        nc.sync.dma_start(nk[:], d["nk"])

        ones_col = consts.tile([128, 1], FP)
        nc.vector.memset(ones_col[:], 1.0)
        ones_pd = consts.tile([128, DH], FP)
        nc.vector.memset(ones_pd[:], 1.0)
        inv_row = consts.tile([1, 128], FP)
        nc.vector.memset(inv_row[:], 1.0 / (M + 1))

        qT = big.tile([128, 2, N], FP)
        kT = big.tile([128, 2, JP], FP)
        vsb = big.tile([128, JT, HG, VW], FP)
        Osb = big.tile([128, 2, N], FP)

        # ---- qT projection: qT[d, i] = tanh(sum_c Wq[c, d] x[i, c]) * qm[i]
        for dc in range(2):
            for ic in range(IC):
                ps = mm_ps.tile([128, 512], FP, tag="mm", name=f"psq{dc}{ic}")
                for cc in range(4):
                    nc.tensor.matmul(
                        ps[:],
                        wq[:, cc, dc * 128:(dc + 1) * 128],
                        xT[:, cc, ic * 512:(ic + 1) * 512],
                        start=(cc == 0), stop=(cc == 3),
                    )
                dst = qT[:, dc, ic * 512:(ic + 1) * 512]
                nc.scalar.activation(dst, ps[:], AF.Tanh)
                nc.vector.tensor_mul(dst, dst, qmB[:, ic * 512:(ic + 1) * 512])

        # ---- kT projection (+ tanh), null col, zero pad
        for dc in range(2):
            for jc in range(IC):
                ps = mm_ps.tile([128, 512], FP, tag="mm", name=f"psk{dc}{jc}")
                for cc in range(4):
                    nc.tensor.matmul(
                        ps[:],
                        wk[:, cc, dc * 128:(dc + 1) * 128],
                        cxT[:, cc, jc * 512:(jc + 1) * 512],
                        start=(cc == 0), stop=(cc == 3),
                    )
                nc.scalar.activation(kT[:, dc, jc * 512:(jc + 1) * 512], ps[:], AF.Tanh)
        nc.vector.memset(kT[:, :, M + 1:JP], 0.0)
        for dc in range(2):
            nc.scalar.activation(kT[:, dc, M:M + 1], nk[:], AF.Tanh)

        # ---- v projection: v[j, d]; last col of each head block = ones (denominator)
        nc.vector.memset(vsb[:, JT - 1, :, :], 0.0)
        for jt in range(JT - 1):
            ps = mm_ps.tile([128, DG], FP, tag="mm", name=f"psv{jt}")
            for cc in range(4):
                nc.tensor.matmul(
                    ps[:],
                    cxT[:, cc, jt * 128:(jt + 1) * 128],
                    wv[:, cc, :],
                    start=(cc == 0), stop=(cc == 3),
                )
            nc.vector.tensor_copy(
                vsb[:, jt, :, 0:DH],
                ps[:].rearrange("p (h e) -> p h e", h=HG),
            )
            nc.vector.memset(vsb[:, jt, :, DH:VW], 1.0)
        # null token row (j = M) lives at partition 0 of the last j tile
        nc.sync.dma_start(vsb[0:1, JT - 1, :, 0:DH],
                          d["nv"].rearrange("a (h e) -> a h e", h=HG))
        nc.vector.memset(vsb[0:1, JT - 1, :, DH:VW], 1.0)

        # ---- correction vectors (masked queries -> uniform attention)
        # corr_h = (scb/2049) * sum_all_j v_aug  -  sum_j exp(cmb_j) v_aug_j
        # (ones column of v_aug makes the denominator slot exactly 0)
        corr = consts.tile([1, HG, VW], FP)
        ps_scb = mm_ps.tile([1, JT], FP, tag="mm")
        nc.tensor.matmul(ps_scb[:], ones_col[:], cmexp[:], start=True, stop=True)
        scbrow = consts.tile([1, JT], FP)
        scb = consts.tile([1, 1], FP)
        nc.scalar.activation(scbrow[:], ps_scb[:], AF.Copy, accum_out=scb[:])
        ps_is = mm_ps.tile([128, 1], FP, tag="mm")
        nc.tensor.matmul(ps_is[:], inv_row[:], scb[:], start=True, stop=True)
        invscb = consts.tile([128, 1], FP)
        nc.scalar.copy(invscb[:], ps_is[:])
        for h in range(HG):
            ps_c = mm_ps.tile([1, VW], FP, tag="mm", name=f"psc{h}")
            for jt in range(JT):
                nc.tensor.matmul(ps_c[:], invscb[:], vsb[:, jt, h, :],
                                 start=(jt == 0), stop=False)
            for jt in range(JT):
                nc.tensor.matmul(ps_c[:], negcm[:, jt:jt + 1], vsb[:, jt, h, :],
                                 start=False, stop=(jt == JT - 1))
            nc.scalar.copy(corr[:, h, :], ps_c[:])

        # ---- flash attention over i chunks
        for ic in range(IC):
            isl = slice(ic * 512, (ic + 1) * 512)
            po = []
            for h in range(HG):
                po.append(acc_ps.tile([128, 512], FP, tag="po", name=f"po{ic}{h}"))
            for jt in range(JT):
                for h in range(HG):
                    pss = mm_ps.tile([128, 512], FP, tag="mm", name=f"pss{ic}{jt}{h}")
                    prow = 64 * (h % 2)
                    nc.tensor.matmul(
                        pss[:],
                        kT[prow:prow + DH, h // 2, jt * 128:(jt + 1) * 128],
                        qT[prow:prow + DH, h // 2, isl],
                        start=True, stop=True,
                    )
                    Ssb = spool.tile([128, 512], FP, tag="s", name=f"s{ic}{jt}{h}")
                    nc.scalar.activation(Ssb[:], pss[:], AF.Exp,
                                         bias=cmb[:, jt:jt + 1], scale=float(SCALE))
                    nc.tensor.matmul(
                        po[h][0:VW, :],
                        vsb[:, jt, h, :],
                        Ssb[:],
                        start=(jt == 0), stop=False,
                    )
            for h in range(HG):
                # rank-1 correction for masked queries (den row gets +0)
                nc.tensor.matmul(
                    po[h][0:VW, :],
                    corr[:, h, :],
                    one_minus_qm[:, isl],
                    start=False, stop=True,
                )
                den = small.tile([128, 512], FP, tag="den")
                nc.vector.tensor_copy(den[DH:VW, :], po[h][DH:VW, :])
                nc.vector.reciprocal(den[DH:VW, :], den[DH:VW, :])
                pr = r_ps.tile([DH, 512], FP, tag="pr", name=f"pr{ic}{h}")
                nc.tensor.matmul(pr[:], ones_pd[DH:VW, 0:DH], den[DH:VW, :],
                                 start=True, stop=True)
                prs = spool.tile([DH, 512], FP, tag="prs", name=f"prs{ic}{h}")
                nc.vector.tensor_copy(prs[:], pr[:])
                if h % 2 == 0:
                    nc.vector.tensor_mul(
                        Osb[0:DH, h // 2, isl], po[h][0:DH, :], prs[:])
                else:
                    ot = small.tile([DH, 512], FP, tag="ot")
                    nc.vector.tensor_mul(ot[:], po[h][0:DH, :], prs[:])
                    nc.sync.dma_start(Osb[DH:128, h // 2, isl], ot[:])

        # ---- output projection: out[i, o] = sum_hd O[hd, i] wo[hd, o]
        for it in range(N // 128):
            pf = mm_ps.tile([128, DIM], FP, tag="mm", name=f"pf{it}")
            for dc in range(2):
                nc.tensor.matmul(
                    pf[:],
                    Osb[:, dc, it * 128:(it + 1) * 128],
                    wo[:, dc, :],
                    start=(dc == 0), stop=(dc == 1),
                )
            fo = spool.tile([128, DIM], FP, tag="fo", name=f"fo{it}")
            nc.vector.tensor_copy(fo[:], pf[:])
            nc.sync.dma_start(d["out"][it * 128:(it + 1) * 128, :], fo[:])


def _core_inputs(inputs, core):
    b, g = core // 2, core % 2
    x = np.asarray(inputs["x"], np.float32)
    context = np.asarray(inputs["context"], np.float32)
    mask = np.asarray(inputs["mask"])
    context_mask = np.asarray(inputs["context_mask"])
    Wq = np.asarray(inputs["Wq"], np.float32)
    Wkv = np.asarray(inputs["Wkv"], np.float32)
    Wo = np.asarray(inputs["Wo"], np.float32)
    null_key = np.asarray(inputs["null_key"], np.float32)
    null_value = np.asarray(inputs["null_value"], np.float32)

    gs = slice(g * DG, (g + 1) * DG)
    cm = np.zeros(JP, np.float32)
    cm[:M] = context_mask[b].astype(np.float32)
    cm[M] = 1.0
    return {
        "xT": np.ascontiguousarray(x[b].T),
        "cxT": np.ascontiguousarray(context[b].T),
        "wq": np.ascontiguousarray(Wq[:, gs]),
        "wk": np.ascontiguousarray(Wkv[:, gs]),
        "wv": np.ascontiguousarray(Wkv[:, DIM + g * DG: DIM + (g + 1) * DG]),
        "wo": np.ascontiguousarray(Wo[gs, :]),
        "qm": mask[b].astype(np.float32).reshape(1, N),
        "cmf": np.ascontiguousarray(cm.reshape(JT, 128).T),
        "nk": np.ascontiguousarray(np.tile(null_key, 2).reshape(128, 1)),
        "nv": np.ascontiguousarray(np.tile(null_value, HG).reshape(1, HG * DH)),
    }


def kernel(x, context, mask, context_mask, Wq, Wkv, Wo, bo, null_key, null_value):
    global LAST_RESULTS
    inputs = {
        "x": x, "context": context, "mask": mask, "context_mask": context_mask,
        "Wq": Wq, "Wkv": Wkv, "Wo": Wo, "bo": bo,
        "null_key": null_key, "null_value": null_value,
    }
    if "nc" not in _CACHE:
        _CACHE["nc"] = _build()
    nc = _CACHE["nc"]
    in_maps = [_core_inputs(inputs, core) for core in range(8)]
    res = bass_utils.run_bass_kernel_spmd(nc, in_maps, core_ids=list(range(8)))
    LAST_RESULTS = res
    bo_np = np.asarray(bo, np.float32)
    out = np.empty((B, N, DIM), np.float32)
    for b in range(B):
        out[b] = res.results[2 * b]["out"] + res.results[2 * b + 1]["out"] + bo_np
    return out



# revision 6
# speedup vs baseline: 5.3961x; 5.3961x over previous
"""Cross-attention kernel for Trainium2, distributed over 8 NeuronCores.

Sharding: data-parallel over batch (4) x tensor-parallel over head groups (2).
Core c handles batch b = c//2, heads [4g, 4g+4) with g = c%2.

Key structural ideas (vs. a dense implementation):

* Host-side compaction. Masked queries (mask[b,i]=False) all produce the
  SAME output row: softmax over an all-masked row is uniform over all
  m+1 positions, so out_i = (sum_j v_j + nv)/(m+1) @ Wo + bo, computable
  on the host for pennies. Masked context positions contribute exactly 0
  after softmax. So the device only sees the ~50% active queries and the
  ~50% unmasked context columns (null token at column 0), cutting the
  quadratic attention work ~4x. Padded to 128-col multiples; pad queries
  are zeros (discarded on host), pad context columns get a -50 logit bias.

* bf16 matmul operands everywhere (weights, x/ctx, q/k after tanh, exp(S),
  O). PE runs 1 cycle/row for bf16 vs 4 for fp32; PSUM accumulation stays
  fp32. Tolerance is 2e-2; bf16 rounding lands ~1e-3.

* Activation-instruction batching: tanh/exp process [128, 2, cs] blocks
  (two head-pair PSUM banks / two d-chunks per instruction) to amortize
  the ~370ns SBUF access latency of the Act engine.

* Softmax denominator: v is augmented with a ones column (row 64 of each
  head's po accumulation = sum of attention weights). The division is a
  DVE reciprocal on the den rows + an SBUF->SBUF broadcast DMA + one DVE
  multiply for all 4 heads at once.

Per-core device pipeline (layouts avoid any on-device transpose):
  qT  = tanh(Wq_g^T @ xc^T)            [256, Npi]  (d on partitions, bf16)
  kT  = tanh(Wk_g^T @ ctxc^T), null col 0          [256, Npj]
  v   = ctxc @ Wv_g (+ nv row 0, ones col)         [Npj, 4x65]
  S   = kT_h^T qT_h per (jtile, headpair, ichunk); exp(0.125 S + bias)
  po_h = v_aug^T @ exp(S)   (row 64 = denominator), divide, O -> Osb
  outc = O^T @ Wo_g                                [Npi, 512]
Host scatters active rows (sum of the two head-group partials + bo) and
fills masked rows with the uniform-attention row.
"""

import numpy as np
import ml_dtypes

import concourse.bass as bass
import concourse.tile as tile
from concourse import bacc, bass_utils, mybir

FP = mybir.dt.float32
BF = mybir.dt.bfloat16
AF = mybir.ActivationFunctionType
NPBF = ml_dtypes.bfloat16

B, N, M, DIM = 4, 2048, 2048, 512
HEADS, DH = 8, 64
INNER = HEADS * DH
G = 2          # head groups (tensor-parallel degree)
HG = 4         # heads per group
DG = HG * DH   # 256 dims per group
NEG = -50.0    # additive pad-mask bias (exp(-50) ~ 2e-22)
SCALE = 1.0 / np.sqrt(DH)  # 0.125
VW = DH + 1    # v columns per head incl. ones column (den row)

LAST_RESULTS = None
LAST_NC = None
_CACHE = {}


def _chunks(total):
    """Split total (multiple of 128) into <=512-sized 128-multiples, as
    equal as possible (each >=256 when total >=512, for full-rate PE)."""
    k = total // 128
    icc = max(1, (total + 511) // 512)
    base, rem = divmod(k, icc)
    out, off = [], 0
    for i in range(icc):
        t = (base + (1 if i < rem else 0)) * 128
        out.append((off, t))
        off += t
    return out


def _build(npi, npj):
    nc = bacc.Bacc("TRN2", debug=False, num_devices=8, enable_partition_id=False)
    d = {}

    def inp(name, shape, dt):
        d[name] = nc.dram_tensor(name, shape, dt, kind="ExternalInput").ap()

    jtc = npj // 128
    inp("xT", [DIM, npi], BF)
    inp("cxT", [DIM, npj], BF)
    inp("wq", [DIM, DG], BF)
    inp("wk", [DIM, DG], BF)
    inp("wv", [DIM, DG], BF)
    inp("wo", [DG, DIM], BF)
    inp("cmb", [128, jtc], FP)   # 0 where attendable, NEG at pad cols
    inp("nk", [128, 1], FP)      # null_key tiled x2
    inp("nv", [1, DG], BF)       # null_value tiled x4
    d["out"] = nc.dram_tensor("out", [npi, DIM], FP, kind="ExternalOutput").ap()

    with tile.TileContext(nc) as tc:
        _body(tc, d, npi, npj)
    nc.compile()
    return nc


def _body(tc, d, npi, npj):
    nc = tc.nc
    jtc = npj // 128
    ichunks = _chunks(npi)
    jchunks = _chunks(npj)

    with (
        tc.tile_pool(name="consts", bufs=1) as consts,
        tc.tile_pool(name="big", bufs=1) as big,
        tc.tile_pool(name="spool", bufs=3) as spool,
        tc.tile_pool(name="fop", bufs=2) as fop,
        tc.tile_pool(name="dpool", bufs=2) as dpool,
        tc.tile_pool(name="sp", bufs=2, space="PSUM") as sp_ps,
        tc.tile_pool(name="acc", bufs=1, space="PSUM") as acc_ps,
    ):
        # ---- constants / inputs ----
        wq = consts.tile([128, 4, DG], BF)
        nc.sync.dma_start(wq[:], d["wq"].rearrange("(c p) d -> p c d", p=128))
        wk = consts.tile([128, 4, DG], BF)
        nc.sync.dma_start(wk[:], d["wk"].rearrange("(c p) d -> p c d", p=128))
        wv = consts.tile([128, 4, DG], BF)
        nc.sync.dma_start(wv[:], d["wv"].rearrange("(c p) d -> p c d", p=128))
        wo = consts.tile([128, 2, DIM], BF)
        nc.sync.dma_start(wo[:], d["wo"].rearrange("(c p) o -> p c o", p=128))
        cmb = consts.tile([128, jtc], FP)
        nc.sync.dma_start(cmb[:], d["cmb"])
        nk = consts.tile([128, 1], FP)
        nc.sync.dma_start(nk[:], d["nk"])
        ones_row = consts.tile([1, DH], BF)
        nc.vector.memset(ones_row[:], 1.0)

        xTc = big.tile([128, 4, npi], BF)
        nc.sync.dma_start(xTc[:], d["xT"].rearrange("(c p) i -> p c i", p=128))
        cxTc = big.tile([128, 4, npj], BF)
        nc.sync.dma_start(cxTc[:], d["cxT"].rearrange("(c p) j -> p c j", p=128))

        qT = big.tile([128, 2, npi], BF)
        kT = big.tile([128, 2, npj], BF)
        vsb = big.tile([128, jtc, HG, VW], BF)
        Osb = big.tile([128, 2, npi], BF)

        nc.vector.memset(vsb[:, :, :, DH:VW], 1.0)

        # ---- q projection: qT[d, i] = tanh(sum_c Wq[c, d] xc[i, c])
        for off, cs in ichunks:
            ps = sp_ps.tile([128, 2, 512], FP, tag="sp", name=f"psq{off}")
            for dc in range(2):
                for cc in range(4):
                    nc.tensor.matmul(
                        ps[:, dc, :cs],
                        wq[:, cc, dc * 128:(dc + 1) * 128],
                        xTc[:, cc, off:off + cs],
                        start=(cc == 0), stop=(cc == 3),
                    )
            nc.scalar.activation(qT[:, :, off:off + cs], ps[:, :, :cs], AF.Tanh)

        # ---- k projection (+ tanh); null-key column 0 overwritten after
        for off, cs in jchunks:
            ps = sp_ps.tile([128, 2, 512], FP, tag="sp", name=f"psk{off}")
            for dc in range(2):
                for cc in range(4):
                    nc.tensor.matmul(
                        ps[:, dc, :cs],
                        wk[:, cc, dc * 128:(dc + 1) * 128],
                        cxTc[:, cc, off:off + cs],
                        start=(cc == 0), stop=(cc == 3),
                    )
            nc.scalar.activation(kT[:, :, off:off + cs], ps[:, :, :cs], AF.Tanh)
        for dc in range(2):
            nc.scalar.activation(kT[:, dc, 0:1], nk[:], AF.Tanh)

        # ---- v projection: v[j, d] per 128-row j tile; two tiles per PSUM buf
        for jt0 in range(0, jtc, 2):
            ps = sp_ps.tile([128, 2, 512], FP, tag="sp", name=f"psv{jt0}")
            for s in range(2):
                jt = jt0 + s
                if jt >= jtc:
                    break
                for cc in range(4):
                    nc.tensor.matmul(
                        ps[:, s, 0:DG],
                        cxTc[:, cc, jt * 128:(jt + 1) * 128],
                        wv[:, cc, :],
                        start=(cc == 0), stop=(cc == 3),
                    )
                nc.vector.tensor_copy(
                    vsb[:, jt, :, 0:DH],
                    ps[:, s, 0:DG].rearrange("p (h e) -> p h e", h=HG),
                )
        # null token value at j=0 (partition 0 of tile 0)
        nc.sync.dma_start(vsb[0:1, 0, :, 0:DH],
                          d["nv"].rearrange("a (h e) -> a h e", h=HG))

        # ---- attention over i chunks; av matmuls pipelined one pair behind
        for off, cs in ichunks:
            po = acc_ps.tile([128, 4, 512], FP, tag="po", name=f"po{off}")

            def emit_av(ssb, jt, hp, _po=po, _cs=cs):
                for hh in range(2):
                    h = 2 * hp + hh
                    nc.tensor.matmul(
                        _po[0:VW, h, :_cs],
                        vsb[:, jt, h, :],
                        ssb[:, hh, :_cs],
                        start=(jt == 0), stop=(jt == jtc - 1),
                    )

            prev = None
            for jt in range(jtc):
                for hp in range(2):
                    sps = sp_ps.tile([128, 2, 512], FP, tag="sp",
                                     name=f"s{off}_{jt}_{hp}")
                    for hh in range(2):
                        nc.tensor.matmul(
                            sps[:, hh, :cs],
                            kT[64 * hh:64 * hh + DH, hp, jt * 128:(jt + 1) * 128],
                            qT[64 * hh:64 * hh + DH, hp, off:off + cs],
                            start=True, stop=True,
                        )
                    ssb = spool.tile([128, 2, 512], BF, tag="s",
                                     name=f"e{off}_{jt}_{hp}")
                    nc.scalar.activation(ssb[:, :, :cs], sps[:, :, :cs], AF.Exp,
                                         bias=cmb[:, jt:jt + 1],
                                         scale=float(SCALE))
                    if prev is not None:
                        emit_av(*prev)
                    prev = (ssb, jt, hp)
            emit_av(*prev)

            # divide by denominator (row DH of each head's po); the 1/den row
            # is broadcast across 64 partitions by a K=1 matmul into PSUM
            posb = dpool.tile([128, 4, 512], FP, tag="posb")
            nc.vector.tensor_copy(posb[0:VW, :, :cs], po[0:VW, :, :cs])
            den_r = dpool.tile([1, 4, 512], BF, tag="den")
            with nc.allow_low_precision(reason="bf16 1/den; rel tol is 2e-2"):
                nc.vector.reciprocal(den_r[:, :, :cs], posb[DH:VW, :, :cs])
            tmpo = dpool.tile([64, 4, 512], BF, tag="tmpo")
            for hp in range(2):
                pr = sp_ps.tile([128, 2, 512], FP, tag="sp", name=f"pr{off}{hp}")
                for s in range(2):
                    nc.tensor.matmul(pr[0:DH, s, :cs], ones_row[:],
                                     den_r[0:1, 2 * hp + s, :cs],
                                     start=True, stop=True)
                nc.vector.tensor_mul(tmpo[:, 2 * hp:2 * hp + 2, :cs],
                                     posb[0:DH, 2 * hp:2 * hp + 2, :cs],
                                     pr[0:DH, :, :cs])
            for h in range(HG):
                nc.sync.dma_start(
                    Osb[64 * (h % 2):64 * (h % 2) + DH, h // 2, off:off + cs],
                    tmpo[:, h, :cs])

            # ---- output projection for this chunk's rows
            for t in range(cs // 128):
                it = off // 128 + t
                pf = sp_ps.tile([128, 2, 512], FP, tag="sp", name=f"pf{it}")
                for dc in range(2):
                    nc.tensor.matmul(
                        pf[:, 0, :],
                        Osb[:, dc, it * 128:(it + 1) * 128],
                        wo[:, dc, :],
                        start=(dc == 0), stop=(dc == 1),
                    )
                fo = fop.tile([128, 512], FP, tag="fo", name=f"fo{it}")
                nc.vector.tensor_copy(fo[:], pf[:, 0, :])
                nc.sync.dma_start(d["out"][it * 128:(it + 1) * 128, :], fo[:])


def _core_inputs(inputs, core, npi, npj, idx_i, idx_j):
    b, g = core // 2, core % 2
    x = np.asarray(inputs["x"], np.float32)
    context = np.asarray(inputs["context"], np.float32)
    Wq = np.asarray(inputs["Wq"], np.float32)
    Wkv = np.asarray(inputs["Wkv"], np.float32)
    Wo = np.asarray(inputs["Wo"], np.float32)
    null_key = np.asarray(inputs["null_key"], np.float32)
    null_value = np.asarray(inputs["null_value"], np.float32)

    ii, jj = idx_i[b], idx_j[b]
    jtc = npj // 128

    xT = np.zeros((DIM, npi), NPBF)
    xT[:, :len(ii)] = x[b][ii].T
    cxT = np.zeros((DIM, npj), NPBF)
    cxT[:, 1:1 + len(jj)] = context[b][jj].T

    cmb = np.full(npj, NEG, np.float32)
    cmb[:1 + len(jj)] = 0.0

    gs = slice(g * DG, (g + 1) * DG)
    return {
        "xT": xT,
        "cxT": cxT,
        "wq": Wq[:, gs].astype(NPBF),
        "wk": Wkv[:, gs].astype(NPBF),
        "wv": Wkv[:, DIM + g * DG: DIM + (g + 1) * DG].astype(NPBF),
        "wo": Wo[gs, :].astype(NPBF),
        "cmb": np.ascontiguousarray(cmb.reshape(jtc, 128).T),
        "nk": np.ascontiguousarray(np.tile(null_key, 2).reshape(128, 1)),
        "nv": np.tile(null_value, HG).reshape(1, DG).astype(NPBF),
    }


def kernel(x, context, mask, context_mask, Wq, Wkv, Wo, bo, null_key, null_value):
    global LAST_RESULTS, LAST_NC
    inputs = {
        "x": x, "context": context, "mask": mask, "context_mask": context_mask,
        "Wq": Wq, "Wkv": Wkv, "Wo": Wo, "bo": bo,
        "null_key": null_key, "null_value": null_value,
    }
    mask_np = np.asarray(mask, bool)
    cm_np = np.asarray(context_mask, bool)
    idx_i = [np.nonzero(mask_np[b])[0] for b in range(B)]
    idx_j = [np.nonzero(cm_np[b])[0] for b in range(B)]
    npi = max(128, -(-max(len(ii) for ii in idx_i) // 128) * 128)
    npj = max(128, -(-max(1 + len(jj) for jj in idx_j) // 128) * 128)

    key = (npi, npj)
    if key not in _CACHE:
        _CACHE[key] = _build(npi, npj)
    nc = _CACHE[key]
    LAST_NC = nc

    in_maps = [_core_inputs(inputs, core, npi, npj, idx_i, idx_j)
               for core in range(8)]
    res = bass_utils.run_bass_kernel_spmd(nc, in_maps, core_ids=list(range(8)))
    LAST_RESULTS = res

    Wkv_np = np.asarray(Wkv, np.float32)
    Wo_np = np.asarray(Wo, np.float32)
    bo_np = np.asarray(bo, np.float32)
    nv_full = np.tile(np.asarray(null_value, np.float32), HEADS)

    out = np.empty((B, N, DIM), np.float32)
    for b in range(B):
        nact = len(idx_i[b])
        if nact:
            s = (res.results[2 * b]["out"][:nact]
                 + res.results[2 * b + 1]["out"][:nact] + bo_np)
            out[b][idx_i[b]] = s
        # masked queries attend uniformly over ALL m+1 positions
        vsum = np.asarray(context[b], np.float32).sum(0) @ Wkv_np[:, INNER:]
        urow = (vsum + nv_full) / (M + 1) @ Wo_np + bo_np
        out[b][~mask_np[b]] = urow
    return out


# revision 9
# speedup vs baseline: 7.0428x; 1.3052x over previous
"""Cross-attention kernel for Trainium2, distributed over 8 NeuronCores.

Sharding: data-parallel over batch (4) x tensor-parallel over head groups (2).
Core c handles batch b = c//2, heads [4g, 4g+4) with g = c%2.

Key structural ideas (vs. a dense implementation):

* Host-side compaction. Masked queries (mask[b,i]=False) all produce the
  SAME output row: softmax over an all-masked row is uniform over all m+1
  positions, so out_i = (sum_j v_j + nv)/(m+1) @ Wo + bo — computed on the
  host. Masked context positions contribute exactly 0 after softmax. The
  device only sees the ~50% active queries and ~50% unmasked context
  columns (null token at column 0), cutting attention work ~4x. Padding
  to 128 multiples: pad queries are zero columns (output discarded); pad
  context columns are zeroed and excluded from softmax by a zero in the
  ones-column of the augmented v (so they add 0 to both numerator and
  denominator — no mask bias needed anywhere).

* bf16 matmul operands everywhere; PSUM accumulation stays fp32. PE runs
  1 cycle/row for bf16 vs 4 for fp32. Tolerance is 2e-2; bf16 lands ~6e-3.

* The attention inner loop is Act-engine-paced (exp is Act-only). Per
  (i-chunk, head-pair): S matmuls and exp run 2 j-tiles ahead of the
  attn@v matmuls (PSUM: 3 score bufs x 2 banks + 1 accumulator x 2 banks),
  so PE never blocks on the S->exp->av latency chain. PE slack inside the
  Act-paced loop is filled with the next i-chunk's q projection and the
  previous i-chunk's output projection.

* A burst of dummy PE matmuls at t=0 keeps the tensor engine busy while
  input DMAs stream, so the p-state ramp (full clock after 3us of
  continuous execution) completes before real work starts.

* Softmax denominator: v is augmented with a ones column (row 64 of each
  head's accumulation). 1/den row -> bf16, broadcast across partitions by
  a K=1 matmul into PSUM, one DVE multiply per head pair.
"""

import numpy as np
import ml_dtypes

import concourse.bass as bass
import concourse.tile as tile
from concourse import bacc, bass_utils, mybir

FP = mybir.dt.float32
BF = mybir.dt.bfloat16
AF = mybir.ActivationFunctionType
NPBF = ml_dtypes.bfloat16

B, N, M, DIM = 4, 2048, 2048, 512
HEADS, DH = 8, 64
INNER = HEADS * DH
G = 2          # head groups (tensor-parallel degree)
HG = 4         # heads per group
DG = HG * DH   # 256 dims per group
SCALE = 1.0 / np.sqrt(DH)  # 0.125
VW = DH + 1    # v columns per head incl. ones column (den row)

LAST_RESULTS = None
LAST_NC = None
_CACHE = {}


def _chunks(total):
    """Split total (multiple of 128) into <=512-sized 128-multiples, as
    equal as possible (each >=256 when total >=512, for full-rate PE)."""
    k = total // 128
    icc = max(1, (total + 511) // 512)
    base, rem = divmod(k, icc)
    out, off = [], 0
    for i in range(icc):
        t = (base + (1 if i < rem else 0)) * 128
        out.append((off, t))
        off += t
    return out


def _build(npi, npj):
    nc = bacc.Bacc("TRN2", debug=False, num_devices=8, enable_partition_id=False)
    d = {}

    def inp(name, shape, dt):
        d[name] = nc.dram_tensor(name, shape, dt, kind="ExternalInput").ap()

    jtc = npj // 128
    inp("xT", [DIM, npi], BF)
    inp("cxT", [DIM, npj], BF)
    inp("wq", [DIM, DG], BF)
    inp("wk", [DIM, DG], BF)
    inp("wv", [DIM, DG], BF)
    inp("wo", [DG, DIM], BF)
    inp("vones", [128, jtc * HG], BF)  # 1 for valid j rows (incl null), 0 pads
    inp("nk", [128, 1], FP)            # null_key tiled x2
    inp("nv", [1, DG], BF)             # null_value tiled x4
    d["out"] = nc.dram_tensor("out", [npi, DIM], FP, kind="ExternalOutput").ap()

    with tile.TileContext(nc) as tc:
        _body(tc, d, npi, npj)
    nc.compile()
    return nc


def _body(tc, d, npi, npj):
    nc = tc.nc
    jtc = npj // 128
    ichunks = _chunks(npi)
    jchunks = _chunks(npj)

    with (
        tc.tile_pool(name="consts", bufs=1) as consts,
        tc.tile_pool(name="big", bufs=1) as big,
        tc.tile_pool(name="spool", bufs=4) as spool,
        tc.tile_pool(name="fop", bufs=2) as fop,
        tc.tile_pool(name="dpool", bufs=2) as dpool,
        tc.tile_pool(name="sp", bufs=3, space="PSUM") as sp_ps,
        tc.tile_pool(name="acc", bufs=1, space="PSUM") as acc_ps,
    ):
        # ---- inputs; ordered so q/k projections unblock earliest ----
        wq = consts.tile([128, 4, DG], BF)
        nc.sync.dma_start(wq[:], d["wq"].rearrange("(c p) d -> p c d", p=128))
        xTc = big.tile([128, 4, npi], BF)
        for off, cs in ichunks:
            nc.sync.dma_start(
                xTc[:, :, off:off + cs],
                d["xT"].rearrange("(c p) i -> p c i", p=128)[:, :, off:off + cs])
        wk = consts.tile([128, 4, DG], BF)
        nc.sync.dma_start(wk[:], d["wk"].rearrange("(c p) d -> p c d", p=128))
        cxTc = big.tile([128, 4, npj], BF)
        for off, cs in jchunks:
            nc.sync.dma_start(
                cxTc[:, :, off:off + cs],
                d["cxT"].rearrange("(c p) j -> p c j", p=128)[:, :, off:off + cs])
        wv = consts.tile([128, 4, DG], BF)
        nc.sync.dma_start(wv[:], d["wv"].rearrange("(c p) d -> p c d", p=128))
        wo = consts.tile([128, 2, DIM], BF)
        nc.sync.dma_start(wo[:], d["wo"].rearrange("(c p) o -> p c o", p=128))
        nk = consts.tile([128, 1], FP)
        nc.sync.dma_start(nk[:], d["nk"])

        qT = big.tile([128, 2, npi], BF)
        kT = big.tile([128, 2, npj], BF)
        vsb = big.tile([128, jtc, HG, VW], BF)
        Osb = big.tile([128, 2, npi], BF)
        nc.sync.dma_start(vsb[:, :, :, DH:VW],
                          d["vones"].rearrange("p (j h) -> p j h", h=HG))

        ones_row = consts.tile([1, DH], BF)
        nc.vector.memset(ones_row[:], 1.0)

        # ---- PE warmup: dummy matmuls keep the p-state ramp going while
        # the input DMAs stream (ramp hits full clock after 3us busy)
        wsrc = consts.tile([128, 256], BF)
        nc.vector.memset(wsrc[:], 0.5)
        wps = sp_ps.tile([128, 2, 512], FP, tag="sp", name="warm")
        for i in range(16):
            nc.tensor.matmul(wps[:, i % 2, 0:256], wsrc[:, 0:128], wsrc[:],
                             start=True, stop=True)

        def qproj(ci):
            off, cs = ichunks[ci]
            ps = sp_ps.tile([128, 2, 512], FP, tag="sp", name=f"psq{off}")
            for dc in range(2):
                for cc in range(4):
                    nc.tensor.matmul(
                        ps[:, dc, :cs],
                        wq[:, cc, dc * 128:(dc + 1) * 128],
                        xTc[:, cc, off:off + cs],
                        start=(cc == 0), stop=(cc == 3),
                    )
            nc.scalar.activation(qT[:, :, off:off + cs], ps[:, :, :cs], AF.Tanh)

        def outproj(ci):
            off, cs = ichunks[ci]
            for t in range(cs // 128):
                it = off // 128 + t
                pf = sp_ps.tile([128, 2, 512], FP, tag="sp", name=f"pf{it}")
                for dc in range(2):
                    nc.tensor.matmul(
                        pf[:, 0, :],
                        Osb[:, dc, it * 128:(it + 1) * 128],
                        wo[:, dc, :],
                        start=(dc == 0), stop=(dc == 1),
                    )
                fo = fop.tile([128, 512], FP, tag="fo", name=f"fo{it}")
                nc.vector.tensor_copy(fo[:], pf[:, 0, :])
                nc.sync.dma_start(d["out"][it * 128:(it + 1) * 128, :], fo[:])

        # ---- projections needed before attention: q chunk 0, all k, all v
        qproj(0)

        for off, cs in jchunks:
            ps = sp_ps.tile([128, 2, 512], FP, tag="sp", name=f"psk{off}")
            for dc in range(2):
                for cc in range(4):
                    nc.tensor.matmul(
                        ps[:, dc, :cs],
                        wk[:, cc, dc * 128:(dc + 1) * 128],
                        cxTc[:, cc, off:off + cs],
                        start=(cc == 0), stop=(cc == 3),
                    )
            nc.scalar.activation(kT[:, :, off:off + cs], ps[:, :, :cs], AF.Tanh)
        for dc in range(2):
            nc.scalar.activation(kT[:, dc, 0:1], nk[:], AF.Tanh)

        for jt0 in range(0, jtc, 2):
            ps = sp_ps.tile([128, 2, 512], FP, tag="sp", name=f"psv{jt0}")
            for s in range(2):
                jt = jt0 + s
                if jt >= jtc:
                    break
                for cc in range(4):
                    nc.tensor.matmul(
                        ps[:, s, 0:DG],
                        cxTc[:, cc, jt * 128:(jt + 1) * 128],
                        wv[:, cc, :],
                        start=(cc == 0), stop=(cc == 3),
                    )
                nc.vector.tensor_copy(
                    vsb[:, jt, :, 0:DH],
                    ps[:, s, 0:DG].rearrange("p (h e) -> p h e", h=HG),
                )
        # null token value at j=0 — must land after the vproj copy of tile 0
        nc.sync.dma_start(vsb[0:1, 0, :, 0:DH],
                          d["nv"].rearrange("a (h e) -> a h e", h=HG))

        # ---- attention: per (i-chunk, head-pair), Act-paced with 2-deep
        # S/exp lookahead; PE slack filled with q/out projections
        nic = len(ichunks)
        for ci in range(nic):
            off, cs = ichunks[ci]
            for hp in range(2):
                po2 = acc_ps.tile([128, 2, 512], FP, tag="po",
                                  name=f"po{ci}{hp}")

                def emit_av(ssb, jt, _po=po2, _cs=cs, _hp=hp):
                    for hh in range(2):
                        nc.tensor.matmul(
                            _po[0:VW, hh, :_cs],
                            vsb[:, jt, 2 * _hp + hh, :],
                            ssb[:, hh, :_cs],
                            start=(jt == 0), stop=(jt == jtc - 1),
                        )

                pend = []
                for jt in range(jtc):
                    sps = sp_ps.tile([128, 2, 512], FP, tag="sp",
                                     name=f"s{ci}_{hp}_{jt}")
                    for hh in range(2):
                        nc.tensor.matmul(
                            sps[:, hh, :cs],
                            kT[64 * hh:64 * hh + DH, hp,
                               jt * 128:(jt + 1) * 128],
                            qT[64 * hh:64 * hh + DH, hp, off:off + cs],
                            start=True, stop=True,
                        )
                    ssb = spool.tile([128, 2, 512], BF, tag="s",
                                     name=f"e{ci}_{hp}_{jt}")
                    nc.scalar.activation(ssb[:, :, :cs], sps[:, :, :cs],
                                         AF.Exp, scale=float(SCALE))
                    pend.append((ssb, jt))
                    if len(pend) > 2:
                        emit_av(*pend.pop(0))

                # PE filler while Act drains the last exps
                if hp == 0 and ci + 1 < nic:
                    qproj(ci + 1)
                if hp == 1 and ci > 0:
                    outproj(ci - 1)
                for args in pend:
                    emit_av(*args)

                # divide by denominator (row DH of each head's po2)
                posb = dpool.tile([128, 2, 512], FP, tag="posb")
                nc.vector.tensor_copy(posb[0:VW, :, :cs], po2[0:VW, :, :cs])
                den_r = dpool.tile([1, 2, 512], BF, tag="den")
                with nc.allow_low_precision(reason="bf16 1/den; tol 2e-2"):
                    nc.vector.reciprocal(den_r[:, :, :cs], posb[DH:VW, :, :cs])
                pr = sp_ps.tile([128, 2, 512], FP, tag="sp", name=f"pr{ci}{hp}")
                for s in range(2):
                    nc.tensor.matmul(pr[0:DH, s, :cs], ones_row[:],
                                     den_r[0:1, s, :cs], start=True, stop=True)
                tmpo = dpool.tile([64, 2, 512], BF, tag="tmpo")
                nc.vector.tensor_mul(tmpo[:, :, :cs], posb[0:DH, :, :cs],
                                     pr[0:DH, :, :cs])
                for s in range(2):
                    nc.sync.dma_start(
                        Osb[64 * s:64 * s + DH, hp, off:off + cs],
                        tmpo[:, s, :cs])

        outproj(nic - 1)


def _core_inputs(inputs, core, npi, npj, idx_i, idx_j):
    b, g = core // 2, core % 2
    x = np.asarray(inputs["x"], np.float32)
    context = np.asarray(inputs["context"], np.float32)
    Wq = np.asarray(inputs["Wq"], np.float32)
    Wkv = np.asarray(inputs["Wkv"], np.float32)
    Wo = np.asarray(inputs["Wo"], np.float32)
    null_key = np.asarray(inputs["null_key"], np.float32)
    null_value = np.asarray(inputs["null_value"], np.float32)

    ii, jj = idx_i[b], idx_j[b]
    jtc = npj // 128

    xT = np.zeros((DIM, npi), NPBF)
    xT[:, :len(ii)] = x[b][ii].T
    cxT = np.zeros((DIM, npj), NPBF)
    cxT[:, 1:1 + len(jj)] = context[b][jj].T

    # validity of each j row (incl. null at 0), replicated per head
    valid = (np.arange(npj) < 1 + len(jj)).astype(np.float32)
    vones = np.repeat(valid.reshape(jtc, 128).T[:, :, None], HG, axis=2)

    gs = slice(g * DG, (g + 1) * DG)
    return {
        "xT": xT,
        "cxT": cxT,
        "wq": Wq[:, gs].astype(NPBF),
        "wk": Wkv[:, gs].astype(NPBF),
        "wv": Wkv[:, DIM + g * DG: DIM + (g + 1) * DG].astype(NPBF),
        "wo": Wo[gs, :].astype(NPBF),
        "vones": np.ascontiguousarray(vones.reshape(128, jtc * HG)).astype(NPBF),
        "nk": np.ascontiguousarray(np.tile(null_key, 2).reshape(128, 1)),
        "nv": np.tile(null_value, HG).reshape(1, DG).astype(NPBF),
    }


def kernel(x, context, mask, context_mask, Wq, Wkv, Wo, bo, null_key, null_value):
    global LAST_RESULTS, LAST_NC
    inputs = {
        "x": x, "context": context, "mask": mask, "context_mask": context_mask,
        "Wq": Wq, "Wkv": Wkv, "Wo": Wo, "bo": bo,
        "null_key": null_key, "null_value": null_value,
    }
    mask_np = np.asarray(mask, bool)
    cm_np = np.asarray(context_mask, bool)
    idx_i = [np.nonzero(mask_np[b])[0] for b in range(B)]
    idx_j = [np.nonzero(cm_np[b])[0] for b in range(B)]
    npi = max(128, -(-max(len(ii) for ii in idx_i) // 128) * 128)
    npj = max(128, -(-max(1 + len(jj) for jj in idx_j) // 128) * 128)

    key = (npi, npj)
    if key not in _CACHE:
        _CACHE[key] = _build(npi, npj)
    nc = _CACHE[key]
    LAST_NC = nc

    in_maps = [_core_inputs(inputs, core, npi, npj, idx_i, idx_j)
               for core in range(8)]
    res = bass_utils.run_bass_kernel_spmd(nc, in_maps, core_ids=list(range(8)))
    LAST_RESULTS = res

    Wkv_np = np.asarray(Wkv, np.float32)
    Wo_np = np.asarray(Wo, np.float32)
    bo_np = np.asarray(bo, np.float32)
    nv_full = np.tile(np.asarray(null_value, np.float32), HEADS)

    out = np.empty((B, N, DIM), np.float32)
    for b in range(B):
        nact = len(idx_i[b])
        if nact:
            s = (res.results[2 * b]["out"][:nact]
                 + res.results[2 * b + 1]["out"][:nact] + bo_np)
            out[b][idx_i[b]] = s
        # masked queries attend uniformly over ALL m+1 positions
        vsum = np.asarray(context[b], np.float32).sum(0) @ Wkv_np[:, INNER:]
        urow = (vsum + nv_full) / (M + 1) @ Wo_np + bo_np
        out[b][~mask_np[b]] = urow
    return out


# revision 21
# speedup vs baseline: 7.5500x; 1.0720x over previous
"""Cross-attention kernel for Trainium2, distributed over 8 NeuronCores.

Sharding: data-parallel over batch (4) x tensor-parallel over head groups (2).
Core c handles batch b = c//2, heads [4g, 4g+4) with g = c%2.

Key structural ideas (vs. a dense implementation):

* Host-side compaction. Masked queries (mask[b,i]=False) all produce the
  SAME output row: softmax over an all-masked row is uniform over all m+1
  positions, so out_i = (sum_j v_j + nv)/(m+1) @ Wo + bo — computed on the
  host. Masked context positions contribute exactly 0 after softmax. The
  device only sees the ~50% active queries and ~50% unmasked context
  columns (null token at column 0), cutting attention work ~4x. Padding
  to 128 multiples: pad queries are zero columns (output discarded); pad
  context columns are zeroed and excluded from softmax by a zero in the
  ones-column of the augmented v (so they add 0 to both numerator and
  denominator — no mask bias needed anywhere).

* bf16 matmul operands everywhere; PSUM accumulation stays fp32. PE runs
  1 cycle/row for bf16 vs 4 for fp32. Tolerance is 2e-2; bf16 lands ~6e-3.

* The attention inner loop is Act-engine-paced (exp is Act-only). Per
  (i-chunk, head-pair): S matmuls and exp run 2 j-tiles ahead of the
  attn@v matmuls (PSUM: 3 score bufs x 2 banks + 1 accumulator x 2 banks),
  so PE never blocks on the S->exp->av latency chain. PE slack inside the
  Act-paced loop is filled with the next i-chunk's q projection and the
  previous i-chunk's output projection.

* A burst of dummy PE matmuls at t=0 keeps the tensor engine busy while
  input DMAs stream, so the p-state ramp (full clock after 3us of
  continuous execution) completes before real work starts.

* Softmax denominator: v is augmented with a ones column (row 64 of each
  head's accumulation). 1/den row -> bf16, broadcast across partitions by
  a K=1 matmul into PSUM, one DVE multiply per head pair.
"""

import numpy as np
import ml_dtypes

import concourse.bass as bass
import concourse.tile as tile
from concourse import bacc, bass_utils, mybir

FP = mybir.dt.float32
BF = mybir.dt.bfloat16
AF = mybir.ActivationFunctionType
NPBF = ml_dtypes.bfloat16

B, N, M, DIM = 4, 2048, 2048, 512
HEADS, DH = 8, 64
INNER = HEADS * DH
G = 2          # head groups (tensor-parallel degree)
HG = 4         # heads per group
DG = HG * DH   # 256 dims per group
SCALE = 1.0 / np.sqrt(DH)  # 0.125
VW = DH + 1    # v columns per head incl. ones column (den row)

LAST_RESULTS = None
LAST_NC = None
_CACHE = {}


_SPLIT_SKIP = (
    "InstDrain", "InstUnconditionalBranch", "InstCall",
    "InstEventSemaphore", "InstRegisterMove", "InstDmaTrigger",
)


def _split_pe_waits(nc):
    """Hoist all-but-one sync waits from compute-engine instructions onto
    fresh same-engine nops placed immediately before them (TRN2 TPB
    instruction structs accept only one sync wait in walrus codegen;
    drains/branches/DMA handle waits differently)."""
    engines = {
        mybir.EngineType.PE: nc.tensor,
        mybir.EngineType.Activation: nc.scalar,
        mybir.EngineType.DVE: nc.vector,
        mybir.EngineType.Pool: nc.gpsimd,
        mybir.EngineType.SP: nc.sync,
    }
    total = 0
    for bb in nc.m.functions[0].blocks:
        new_insts = []
        for ins in bb.instructions:
            si = ins.sync_info
            eng = engines.get(getattr(ins, "engine", None))
            if (
                eng is not None
                and type(ins).__name__ not in _SPLIT_SKIP
                and si is not None
                and si.on_wait
                and len(si.on_wait) > 1
            ):
                waits = list(si.on_wait)
                for w in waits[:-1]:
                    nop = eng._isa(
                        nc.isa.Opcode.NEURON_ISA_TPB_OPCODE_ENGINE_NOP,
                        {}, None, [], [], True,
                    )
                    nop.sync_info = mybir.SyncInfo(on_wait=[w], on_update=[])
                    nc.inst_map[nop.name] = nop
                    new_insts.append(nop)
                    total += 1
                si.on_wait = waits[-1:]
            new_insts.append(ins)
        bb.instructions = new_insts
    return total


def _chunks(total):
    """Split total (multiple of 128) into <=512-sized 128-multiples, as
    equal as possible (each >=256 when total >=512, for full-rate PE)."""
    k = total // 128
    icc = max(1, (total + 511) // 512)
    base, rem = divmod(k, icc)
    out, off = [], 0
    for i in range(icc):
        t = (base + (1 if i < rem else 0)) * 128
        out.append((off, t))
        off += t
    return out


def _build(npi, npj):
    nc = bacc.Bacc("TRN2", debug=False, num_devices=8, enable_partition_id=False)
    d = {}

    def inp(name, shape, dt):
        d[name] = nc.dram_tensor(name, shape, dt, kind="ExternalInput").ap()

    jtc = npj // 128
    inp("xT", [DIM, npi], BF)
    inp("cxT", [DIM, npj], BF)
    inp("wq", [DIM, DG], BF)
    inp("wk", [DIM, DG], BF)
    inp("wv", [DIM, DG], BF)
    inp("wo", [DG, DIM], BF)
    inp("vones", [128, jtc * HG], BF)  # 1 for valid j rows (incl null), 0 pads
    inp("nk", [128, 1], FP)            # null_key tiled x2
    inp("nv", [1, DG], BF)             # null_value tiled x4
    d["out"] = nc.dram_tensor("out", [npi, DIM], FP, kind="ExternalOutput").ap()

    with tile.TileContext(nc) as tc:
        _body(tc, d, npi, npj)
    nc.compile()
    return nc


def _body(tc, d, npi, npj):
    nc = tc.nc
    jtc = npj // 128
    ichunks = _chunks(npi)
    jchunks = _chunks(npj)

    with (
        tc.tile_pool(name="consts", bufs=1) as consts,
        tc.tile_pool(name="big", bufs=1) as big,
        tc.tile_pool(name="spool", bufs=4) as spool,
        tc.tile_pool(name="fop", bufs=2) as fop,
        tc.tile_pool(name="dpool", bufs=2) as dpool,
        tc.tile_pool(name="sp", bufs=3, space="PSUM") as sp_ps,
        tc.tile_pool(name="acc", bufs=1, space="PSUM") as acc_ps,
    ):
        # ---- inputs; ordered so the k projection (first consumer after
        # warmup) unblocks earliest
        wk = consts.tile([128, 4, DG], BF)
        nc.sync.dma_start(wk[:], d["wk"].rearrange("(c p) d -> p c d", p=128))
        cxTc = big.tile([128, 4, npj], BF)
        for off, cs in jchunks:
            nc.sync.dma_start(
                cxTc[:, :, off:off + cs],
                d["cxT"].rearrange("(c p) j -> p c j", p=128)[:, :, off:off + cs])
        wq = consts.tile([128, 4, DG], BF)
        nc.sync.dma_start(wq[:], d["wq"].rearrange("(c p) d -> p c d", p=128))
        xTc = big.tile([128, 4, npi], BF)
        for off, cs in ichunks:
            nc.sync.dma_start(
                xTc[:, :, off:off + cs],
                d["xT"].rearrange("(c p) i -> p c i", p=128)[:, :, off:off + cs])
        wv = consts.tile([128, 4, DG], BF)
        nc.sync.dma_start(wv[:], d["wv"].rearrange("(c p) d -> p c d", p=128))
        wo = consts.tile([128, 2, DIM], BF)
        nc.sync.dma_start(wo[:], d["wo"].rearrange("(c p) o -> p c o", p=128))
        nk = consts.tile([128, 1], FP)
        nc.sync.dma_start(nk[:], d["nk"])

        qT = big.tile([128, 2, npi], BF)
        kT = big.tile([128, 2, npj], BF)
        vsb = big.tile([128, jtc, HG, VW], BF)
        Osb = big.tile([128, 2, npi], BF)
        nc.sync.dma_start(vsb[:, :, :, DH:VW],
                          d["vones"].rearrange("p (j h) -> p j h", h=HG))

        ones_row = consts.tile([1, DH], BF)
        nc.vector.memset(ones_row[:], 1.0)

        # ---- PE warmup: dummy matmuls keep the p-state ramp going while
        # the input DMAs stream (ramp hits full clock after 3us busy)
        wsrc = consts.tile([128, 256], BF)
        nc.vector.memset(wsrc[:], 0.5)
        wps = sp_ps.tile([128, 2, 512], FP, tag="sp", name="warm")
        for i in range(10):
            nc.tensor.matmul(wps[:, i % 2, 0:256], wsrc[:, 0:128], wsrc[:],
                             start=True, stop=True)

        def qproj(ci):
            off, cs = ichunks[ci]
            ps = sp_ps.tile([128, 2, 512], FP, tag="sp", name=f"psq{off}")
            for dc in range(2):
                for cc in range(4):
                    nc.tensor.matmul(
                        ps[:, dc, :cs],
                        wq[:, cc, dc * 128:(dc + 1) * 128],
                        xTc[:, cc, off:off + cs],
                        start=(cc == 0), stop=(cc == 3),
                    )
            nc.scalar.activation(qT[:, :, off:off + cs], ps[:, :, :cs], AF.Tanh)

        def outproj(ci):
            off, cs = ichunks[ci]
            for t in range(cs // 128):
                it = off // 128 + t
                pf = sp_ps.tile([128, 2, 512], FP, tag="sp", name=f"pf{it}")
                for dc in range(2):
                    nc.tensor.matmul(
                        pf[:, 0, :],
                        Osb[:, dc, it * 128:(it + 1) * 128],
                        wo[:, dc, :],
                        start=(dc == 0), stop=(dc == 1),
                    )
                fo = fop.tile([128, 512], FP, tag="fo", name=f"fo{it}")
                nc.vector.tensor_copy(fo[:], pf[:, 0, :])
                nc.sync.dma_start(d["out"][it * 128:(it + 1) * 128, :], fo[:])

        # ---- projections needed before attention: all k, q chunk 0, all v
        for off, cs in jchunks:
            ps = sp_ps.tile([128, 2, 512], FP, tag="sp", name=f"psk{off}")
            for dc in range(2):
                for cc in range(4):
                    nc.tensor.matmul(
                        ps[:, dc, :cs],
                        wk[:, cc, dc * 128:(dc + 1) * 128],
                        cxTc[:, cc, off:off + cs],
                        start=(cc == 0), stop=(cc == 3),
                    )
            nc.scalar.activation(kT[:, :, off:off + cs], ps[:, :, :cs], AF.Tanh)
        for dc in range(2):
            nc.scalar.activation(kT[:, dc, 0:1], nk[:], AF.Tanh)

        qproj(0)

        for jt0 in range(0, jtc, 2):
            ps = sp_ps.tile([128, 2, 512], FP, tag="sp", name=f"psv{jt0}")
            for s in range(2):
                jt = jt0 + s
                if jt >= jtc:
                    break
                for cc in range(4):
                    nc.tensor.matmul(
                        ps[:, s, 0:DG],
                        cxTc[:, cc, jt * 128:(jt + 1) * 128],
                        wv[:, cc, :],
                        start=(cc == 0), stop=(cc == 3),
                    )
                nc.vector.tensor_copy(
                    vsb[:, jt, :, 0:DH],
                    ps[:, s, 0:DG].rearrange("p (h e) -> p h e", h=HG),
                )
        # null token value at j=0 — must land after the vproj copy of tile 0
        nc.sync.dma_start(vsb[0:1, 0, :, 0:DH],
                          d["nv"].rearrange("a (h e) -> a h e", h=HG))

        # ---- attention: one pipelined stream over (i-chunk, head-pair)
        # segments x j tiles. S matmuls + exp run 2 j-tiles ahead of the
        # attn@v matmuls, ACROSS segment boundaries, so the PE never sits
        # through the Act engine's exp drain at a segment's tail. Each
        # segment's denominator division is emitted right after its last
        # attn@v (i.e. inside the next segment's stream); PE slack inside
        # the Act-paced loop is filled with q/out projections.
        nic = len(ichunks)
        segs = [(ci, hp) for ci in range(nic) for hp in range(2)]
        po_of = {}

        def emit_av(item):
            ssb, jt, ci, hp = item
            off, cs = ichunks[ci]
            if jt == 0:  # lazily created so pool-buffer order == use order
                po_of[(ci, hp)] = acc_ps.tile([128, 2, 512], FP, tag="po",
                                              name=f"po{ci}{hp}")
            po2 = po_of[(ci, hp)]
            for hh in range(2):
                nc.tensor.matmul(
                    po2[0:VW, hh, :cs],
                    vsb[:, jt, 2 * hp + hh, :],
                    ssb[:, hh, :cs],
                    start=(jt == 0), stop=(jt == jtc - 1),
                )
            if jt == jtc - 1:
                den_div(ci, hp)

        def den_div(ci, hp):
            # divide by denominator (row DH of each head's po2)
            off, cs = ichunks[ci]
            po2 = po_of[(ci, hp)]
            posb = dpool.tile([128, 2, 512], FP, tag="posb")
            nc.vector.tensor_copy(posb[0:VW, :, :cs], po2[0:VW, :, :cs])
            den_r = dpool.tile([1, 2, 512], BF, tag="den")
            with nc.allow_low_precision(reason="bf16 1/den; tol 2e-2"):
                nc.vector.reciprocal(den_r[:, :, :cs], posb[DH:VW, :, :cs])
            pr = sp_ps.tile([128, 2, 512], FP, tag="sp", name=f"pr{ci}{hp}")
            for s in range(2):
                nc.tensor.matmul(pr[0:DH, s, :cs], ones_row[:],
                                 den_r[0:1, s, :cs], start=True, stop=True)
            tmpo = dpool.tile([64, 2, 512], BF, tag="tmpo")
            nc.vector.tensor_mul(tmpo[:, :, :cs], posb[0:DH, :, :cs],
                                 pr[0:DH, :, :cs])
            for s in range(2):
                nc.sync.dma_start(
                    Osb[64 * s:64 * s + DH, hp, off:off + cs],
                    tmpo[:, s, :cs])

        pend = []
        for ci, hp in segs:
            off, cs = ichunks[ci]
            for jt in range(jtc):
                sps = sp_ps.tile([128, 2, 512], FP, tag="sp",
                                 name=f"s{ci}_{hp}_{jt}")
                for hh in range(2):
                    nc.tensor.matmul(
                        sps[:, hh, :cs],
                        kT[64 * hh:64 * hh + DH, hp, jt * 128:(jt + 1) * 128],
                        qT[64 * hh:64 * hh + DH, hp, off:off + cs],
                        start=True, stop=True,
                    )
                ssb = spool.tile([128, 2, 512], BF, tag="s",
                                 name=f"e{ci}_{hp}_{jt}")
                nc.scalar.activation(ssb[:, :, :cs], sps[:, :, :cs],
                                     AF.Exp, scale=float(SCALE))
                pend.append((ssb, jt, ci, hp))
                if len(pend) > 2:
                    emit_av(pend.pop(0))
            # PE filler between segments while Act drains pending exps
            if hp == 0 and ci + 1 < nic:
                qproj(ci + 1)
            if hp == 1 and ci > 0:
                outproj(ci - 1)
        while pend:
            emit_av(pend.pop(0))
        outproj(nic - 1)


def _core_inputs(inputs, core, npi, npj, idx_i, idx_j):
    b, g = core // 2, core % 2
    x = np.asarray(inputs["x"], np.float32)
    context = np.asarray(inputs["context"], np.float32)
    Wq = np.asarray(inputs["Wq"], np.float32)
    Wkv = np.asarray(inputs["Wkv"], np.float32)
    Wo = np.asarray(inputs["Wo"], np.float32)
    null_key = np.asarray(inputs["null_key"], np.float32)
    null_value = np.asarray(inputs["null_value"], np.float32)

    ii, jj = idx_i[b], idx_j[b]
    jtc = npj // 128

    xT = np.zeros((DIM, npi), NPBF)
    xT[:, :len(ii)] = x[b][ii].T
    cxT = np.zeros((DIM, npj), NPBF)
    cxT[:, 1:1 + len(jj)] = context[b][jj].T

    # validity of each j row (incl. null at 0), replicated per head
    valid = (np.arange(npj) < 1 + len(jj)).astype(np.float32)
    vones = np.repeat(valid.reshape(jtc, 128).T[:, :, None], HG, axis=2)

    gs = slice(g * DG, (g + 1) * DG)
    return {
        "xT": xT,
        "cxT": cxT,
        "wq": Wq[:, gs].astype(NPBF),
        "wk": Wkv[:, gs].astype(NPBF),
        "wv": Wkv[:, DIM + g * DG: DIM + (g + 1) * DG].astype(NPBF),
        "wo": Wo[gs, :].astype(NPBF),
        "vones": np.ascontiguousarray(vones.reshape(128, jtc * HG)).astype(NPBF),
        "nk": np.ascontiguousarray(np.tile(null_key, 2).reshape(128, 1)),
        "nv": np.tile(null_value, HG).reshape(1, DG).astype(NPBF),
    }


def kernel(x, context, mask, context_mask, Wq, Wkv, Wo, bo, null_key, null_value):
    global LAST_RESULTS, LAST_NC
    inputs = {
        "x": x, "context": context, "mask": mask, "context_mask": context_mask,
        "Wq": Wq, "Wkv": Wkv, "Wo": Wo, "bo": bo,
        "null_key": null_key, "null_value": null_value,
    }
    mask_np = np.asarray(mask, bool)
    cm_np = np.asarray(context_mask, bool)
    idx_i = [np.nonzero(mask_np[b])[0] for b in range(B)]
    idx_j = [np.nonzero(cm_np[b])[0] for b in range(B)]
    npi = max(128, -(-max(len(ii) for ii in idx_i) // 128) * 128)
    npj = max(128, -(-max(1 + len(jj) for jj in idx_j) // 128) * 128)

    key = (npi, npj)
    if key not in _CACHE:
        _CACHE[key] = _build(npi, npj)
    nc = _CACHE[key]
    LAST_NC = nc

    in_maps = [_core_inputs(inputs, core, npi, npj, idx_i, idx_j)
               for core in range(8)]
    res = bass_utils.run_bass_kernel_spmd(nc, in_maps, core_ids=list(range(8)))
    LAST_RESULTS = res

    Wkv_np = np.asarray(Wkv, np.float32)
    Wo_np = np.asarray(Wo, np.float32)
    bo_np = np.asarray(bo, np.float32)
    nv_full = np.tile(np.asarray(null_value, np.float32), HEADS)

    out = np.empty((B, N, DIM), np.float32)
    for b in range(B):
        nact = len(idx_i[b])
        if nact:
            s = (res.results[2 * b]["out"][:nact]
                 + res.results[2 * b + 1]["out"][:nact] + bo_np)
            out[b][idx_i[b]] = s
        # masked queries attend uniformly over ALL m+1 positions
        vsum = np.asarray(context[b], np.float32).sum(0) @ Wkv_np[:, INNER:]
        urow = (vsum + nv_full) / (M + 1) @ Wo_np + bo_np
        out[b][~mask_np[b]] = urow
    return out


# revision 23
# speedup vs baseline: 7.5555x; 1.0007x over previous
"""Cross-attention kernel for Trainium2, distributed over 8 NeuronCores.

Sharding: data-parallel over batch (4) x tensor-parallel over head groups (2).
Core c handles batch b = c//2, heads [4g, 4g+4) with g = c%2.

Key structural ideas (vs. a dense implementation):

* Host-side compaction. Masked queries (mask[b,i]=False) all produce the
  SAME output row: softmax over an all-masked row is uniform over all m+1
  positions, so out_i = (sum_j v_j + nv)/(m+1) @ Wo + bo — computed on the
  host. Masked context positions contribute exactly 0 after softmax. The
  device only sees the ~50% active queries and ~50% unmasked context
  columns (null token at column 0), cutting attention work ~4x. Padding
  to 128 multiples: pad queries are zero columns (output discarded); pad
  context columns are zeroed and excluded from softmax by a zero in the
  ones-column of the augmented v (so they add 0 to both numerator and
  denominator — no mask bias needed anywhere).

* bf16 matmul operands everywhere; PSUM accumulation stays fp32. PE runs
  1 cycle/row for bf16 vs 4 for fp32. Tolerance is 2e-2; bf16 lands ~6e-3.

* The attention inner loop is Act-engine-paced (exp is Act-only). Per
  (i-chunk, head-pair): S matmuls and exp run 2 j-tiles ahead of the
  attn@v matmuls (PSUM: 3 score bufs x 2 banks + 1 accumulator x 2 banks),
  so PE never blocks on the S->exp->av latency chain. PE slack inside the
  Act-paced loop is filled with the next i-chunk's q projection and the
  previous i-chunk's output projection.

* A burst of dummy PE matmuls at t=0 keeps the tensor engine busy while
  input DMAs stream, so the p-state ramp (full clock after 3us of
  continuous execution) completes before real work starts.

* Softmax denominator: v is augmented with a ones column (row 64 of each
  head's accumulation). 1/den row -> bf16, broadcast across partitions by
  a K=1 matmul into PSUM, one DVE multiply per head pair.
"""

import numpy as np
import ml_dtypes

import concourse.bass as bass
import concourse.tile as tile
from concourse import bacc, bass_utils, mybir

FP = mybir.dt.float32
BF = mybir.dt.bfloat16
AF = mybir.ActivationFunctionType
NPBF = ml_dtypes.bfloat16

B, N, M, DIM = 4, 2048, 2048, 512
HEADS, DH = 8, 64
INNER = HEADS * DH
G = 2          # head groups (tensor-parallel degree)
HG = 4         # heads per group
DG = HG * DH   # 256 dims per group
SCALE = 1.0 / np.sqrt(DH)  # 0.125
VW = DH + 1    # v columns per head incl. ones column (den row)

LAST_RESULTS = None
LAST_NC = None
_CACHE = {}


_SPLIT_SKIP = (
    "InstDrain", "InstUnconditionalBranch", "InstCall",
    "InstEventSemaphore", "InstRegisterMove", "InstDmaTrigger",
)


def _split_multi_waits(nc):
    """TRN2 TPB instruction structs accept only ONE sync wait in walrus
    codegen; extra waits assigned by the Tile scheduler are silently dropped
    from the NEFF, which races on hardware. Hoist all-but-one wait onto
    standalone same-engine InstEventSemaphore instructions (sequencer-only
    waits, the same mechanism the framework itself uses) placed immediately
    before the offending instruction."""
    valid = set(mybir.EngineType) - {mybir.EngineType.Unassigned}
    total = 0
    for bb in nc.m.functions[0].blocks:
        new_insts = []
        for ins in bb.instructions:
            si = ins.sync_info
            if (
                getattr(ins, "engine", None) in valid
                and type(ins).__name__ not in _SPLIT_SKIP
                and si is not None
                and si.on_wait
                and len(si.on_wait) > 1
            ):
                waits = list(si.on_wait)
                for w in waits[:-1]:
                    total += 1
                    ev = mybir.InstEventSemaphore(
                        name=f"evsplit{total}_{ins.name}", ins=[], outs=[])
                    ev.engine = ins.engine
                    ev.sync_info = mybir.SyncInfo(on_wait=[w], on_update=[])
                    nc.inst_map[ev.name] = ev
                    new_insts.append(ev)
                si.on_wait = waits[-1:]
            new_insts.append(ins)
        bb.instructions = new_insts
    return total


def _chunks(total):
    """Split total (multiple of 128) into <=512-sized 128-multiples, as
    equal as possible (each >=256 when total >=512, for full-rate PE)."""
    k = total // 128
    icc = max(1, (total + 511) // 512)
    base, rem = divmod(k, icc)
    out, off = [], 0
    for i in range(icc):
        t = (base + (1 if i < rem else 0)) * 128
        out.append((off, t))
        off += t
    return out


def _build(npi, npj):
    nc = bacc.Bacc("TRN2", debug=False, num_devices=8, enable_partition_id=False)
    d = {}

    def inp(name, shape, dt):
        d[name] = nc.dram_tensor(name, shape, dt, kind="ExternalInput").ap()

    jtc = npj // 128
    inp("xT", [DIM, npi], BF)
    inp("cxT", [DIM, npj], BF)
    inp("wq", [DIM, DG], BF)
    inp("wk", [DIM, DG], BF)
    inp("wv", [DIM, DG], BF)
    inp("wo", [DG, DIM], BF)
    inp("vones", [128, jtc * HG], BF)  # 1 for valid j rows (incl null), 0 pads
    inp("nk", [128, 1], FP)            # null_key tiled x2
    inp("nv", [1, DG], BF)             # null_value tiled x4
    d["out"] = nc.dram_tensor("out", [npi, DIM], FP, kind="ExternalOutput").ap()

    with tile.TileContext(nc) as tc:
        _body(tc, d, npi, npj)
    _split_multi_waits(nc)
    nc.compile()
    return nc


def _body(tc, d, npi, npj):
    nc = tc.nc
    jtc = npj // 128
    ichunks = _chunks(npi)
    jchunks = _chunks(npj)

    with (
        tc.tile_pool(name="consts", bufs=1) as consts,
        tc.tile_pool(name="big", bufs=1) as big,
        tc.tile_pool(name="spool", bufs=4) as spool,
        tc.tile_pool(name="fop", bufs=2) as fop,
        tc.tile_pool(name="dpool", bufs=2) as dpool,
        tc.tile_pool(name="sp", bufs=3, space="PSUM") as sp_ps,
        tc.tile_pool(name="acc", bufs=1, space="PSUM") as acc_ps,
    ):
        # ---- inputs; ordered so the k projection (first consumer after
        # warmup) unblocks earliest
        wk = consts.tile([128, 4, DG], BF)
        nc.sync.dma_start(wk[:], d["wk"].rearrange("(c p) d -> p c d", p=128))
        cxTc = big.tile([128, 4, npj], BF)
        for off, cs in jchunks:
            nc.sync.dma_start(
                cxTc[:, :, off:off + cs],
                d["cxT"].rearrange("(c p) j -> p c j", p=128)[:, :, off:off + cs])
        wq = consts.tile([128, 4, DG], BF)
        nc.sync.dma_start(wq[:], d["wq"].rearrange("(c p) d -> p c d", p=128))
        xTc = big.tile([128, 4, npi], BF)
        for off, cs in ichunks:
            nc.sync.dma_start(
                xTc[:, :, off:off + cs],
                d["xT"].rearrange("(c p) i -> p c i", p=128)[:, :, off:off + cs])
        wv = consts.tile([128, 4, DG], BF)
        nc.sync.dma_start(wv[:], d["wv"].rearrange("(c p) d -> p c d", p=128))
        wo = consts.tile([128, 2, DIM], BF)
        nc.sync.dma_start(wo[:], d["wo"].rearrange("(c p) o -> p c o", p=128))
        nk = consts.tile([128, 1], FP)
        nc.sync.dma_start(nk[:], d["nk"])

        qT = big.tile([128, 2, npi], BF)
        kT = big.tile([128, 2, npj], BF)
        vsb = big.tile([128, jtc, HG, VW], BF)
        Osb = big.tile([128, 2, npi], BF)
        nc.sync.dma_start(vsb[:, :, :, DH:VW],
                          d["vones"].rearrange("p (j h) -> p j h", h=HG))

        ones_row = consts.tile([1, DH], BF)
        nc.vector.memset(ones_row[:], 1.0)

        # ---- PE warmup: dummy matmuls keep the p-state ramp going while
        # the input DMAs stream (ramp hits full clock after 3us busy)
        wsrc = consts.tile([128, 256], BF)
        nc.vector.memset(wsrc[:], 0.5)
        wps = sp_ps.tile([128, 2, 512], FP, tag="sp", name="warm")
        for i in range(10):
            nc.tensor.matmul(wps[:, i % 2, 0:256], wsrc[:, 0:128], wsrc[:],
                             start=True, stop=True)

        def qproj(ci):
            off, cs = ichunks[ci]
            ps = sp_ps.tile([128, 2, 512], FP, tag="sp", name=f"psq{off}")
            for dc in range(2):
                for cc in range(4):
                    nc.tensor.matmul(
                        ps[:, dc, :cs],
                        wq[:, cc, dc * 128:(dc + 1) * 128],
                        xTc[:, cc, off:off + cs],
                        start=(cc == 0), stop=(cc == 3),
                    )
            nc.scalar.activation(qT[:, :, off:off + cs], ps[:, :, :cs], AF.Tanh)

        def outproj(ci):
            off, cs = ichunks[ci]
            for t in range(cs // 128):
                it = off // 128 + t
                pf = sp_ps.tile([128, 2, 512], FP, tag="sp", name=f"pf{it}")
                for dc in range(2):
                    nc.tensor.matmul(
                        pf[:, 0, :],
                        Osb[:, dc, it * 128:(it + 1) * 128],
                        wo[:, dc, :],
                        start=(dc == 0), stop=(dc == 1),
                    )
                fo = fop.tile([128, 512], FP, tag="fo", name=f"fo{it}")
                nc.vector.tensor_copy(fo[:], pf[:, 0, :])
                nc.sync.dma_start(d["out"][it * 128:(it + 1) * 128, :], fo[:])

        # ---- projections needed before attention: all k, q chunk 0, all v
        for off, cs in jchunks:
            ps = sp_ps.tile([128, 2, 512], FP, tag="sp", name=f"psk{off}")
            for dc in range(2):
                for cc in range(4):
                    nc.tensor.matmul(
                        ps[:, dc, :cs],
                        wk[:, cc, dc * 128:(dc + 1) * 128],
                        cxTc[:, cc, off:off + cs],
                        start=(cc == 0), stop=(cc == 3),
                    )
            nc.scalar.activation(kT[:, :, off:off + cs], ps[:, :, :cs], AF.Tanh)
        for dc in range(2):
            nc.scalar.activation(kT[:, dc, 0:1], nk[:], AF.Tanh)

        qproj(0)

        for jt0 in range(0, jtc, 2):
            ps = sp_ps.tile([128, 2, 512], FP, tag="sp", name=f"psv{jt0}")
            for s in range(2):
                jt = jt0 + s
                if jt >= jtc:
                    break
                for cc in range(4):
                    nc.tensor.matmul(
                        ps[:, s, 0:DG],
                        cxTc[:, cc, jt * 128:(jt + 1) * 128],
                        wv[:, cc, :],
                        start=(cc == 0), stop=(cc == 3),
                    )
                nc.vector.tensor_copy(
                    vsb[:, jt, :, 0:DH],
                    ps[:, s, 0:DG].rearrange("p (h e) -> p h e", h=HG),
                )
        # null token value at j=0 — must land after the vproj copy of tile 0
        nc.sync.dma_start(vsb[0:1, 0, :, 0:DH],
                          d["nv"].rearrange("a (h e) -> a h e", h=HG))

        # ---- attention: one pipelined stream over (i-chunk, head-pair)
        # segments x j tiles. S matmuls + exp run 2 j-tiles ahead of the
        # attn@v matmuls, ACROSS segment boundaries, so the PE never sits
        # through the Act engine's exp drain at a segment's tail. Each
        # segment's denominator division is emitted right after its last
        # attn@v (i.e. inside the next segment's stream); PE slack inside
        # the Act-paced loop is filled with q/out projections.
        nic = len(ichunks)
        segs = [(ci, hp) for ci in range(nic) for hp in range(2)]
        po_of = {}

        def emit_av(item):
            ssb, jt, ci, hp = item
            off, cs = ichunks[ci]
            if jt == 0:  # lazily created so pool-buffer order == use order
                po_of[(ci, hp)] = acc_ps.tile([128, 2, 512], FP, tag="po",
                                              name=f"po{ci}{hp}")
            po2 = po_of[(ci, hp)]
            for hh in range(2):
                nc.tensor.matmul(
                    po2[0:VW, hh, :cs],
                    vsb[:, jt, 2 * hp + hh, :],
                    ssb[:, hh, :cs],
                    start=(jt == 0), stop=(jt == jtc - 1),
                )
            if jt == jtc - 1:
                den_div(ci, hp)

        def den_div(ci, hp):
            # divide by denominator (row DH of each head's po2)
            off, cs = ichunks[ci]
            po2 = po_of[(ci, hp)]
            posb = dpool.tile([128, 2, 512], FP, tag="posb")
            nc.vector.tensor_copy(posb[0:VW, :, :cs], po2[0:VW, :, :cs])
            den_r = dpool.tile([1, 2, 512], BF, tag="den")
            with nc.allow_low_precision(reason="bf16 1/den; tol 2e-2"):
                nc.vector.reciprocal(den_r[:, :, :cs], posb[DH:VW, :, :cs])
            pr = sp_ps.tile([128, 2, 512], FP, tag="sp", name=f"pr{ci}{hp}")
            for s in range(2):
                nc.tensor.matmul(pr[0:DH, s, :cs], ones_row[:],
                                 den_r[0:1, s, :cs], start=True, stop=True)
            tmpo = dpool.tile([64, 2, 512], BF, tag="tmpo")
            nc.vector.tensor_mul(tmpo[:, :, :cs], posb[0:DH, :, :cs],
                                 pr[0:DH, :, :cs])
            for s in range(2):
                nc.sync.dma_start(
                    Osb[64 * s:64 * s + DH, hp, off:off + cs],
                    tmpo[:, s, :cs])

        pend = []
        for ci, hp in segs:
            off, cs = ichunks[ci]
            for jt in range(jtc):
                sps = sp_ps.tile([128, 2, 512], FP, tag="sp",
                                 name=f"s{ci}_{hp}_{jt}")
                for hh in range(2):
                    nc.tensor.matmul(
                        sps[:, hh, :cs],
                        kT[64 * hh:64 * hh + DH, hp, jt * 128:(jt + 1) * 128],
                        qT[64 * hh:64 * hh + DH, hp, off:off + cs],
                        start=True, stop=True,
                    )
                ssb = spool.tile([128, 2, 512], BF, tag="s",
                                 name=f"e{ci}_{hp}_{jt}")
                nc.scalar.activation(ssb[:, :, :cs], sps[:, :, :cs],
                                     AF.Exp, scale=float(SCALE))
                pend.append((ssb, jt, ci, hp))
                if len(pend) > 2:
                    emit_av(pend.pop(0))
            # PE filler between segments while Act drains pending exps
            if hp == 0 and ci + 1 < nic:
                qproj(ci + 1)
            if hp == 1 and ci > 0:
                outproj(ci - 1)
        while pend:
            emit_av(pend.pop(0))
        outproj(nic - 1)


def _core_inputs(inputs, core, npi, npj, idx_i, idx_j):
    b, g = core // 2, core % 2
    x = np.asarray(inputs["x"], np.float32)
    context = np.asarray(inputs["context"], np.float32)
    Wq = np.asarray(inputs["Wq"], np.float32)
    Wkv = np.asarray(inputs["Wkv"], np.float32)
    Wo = np.asarray(inputs["Wo"], np.float32)
    null_key = np.asarray(inputs["null_key"], np.float32)
    null_value = np.asarray(inputs["null_value"], np.float32)

    ii, jj = idx_i[b], idx_j[b]
    jtc = npj // 128

    xT = np.zeros((DIM, npi), NPBF)
    xT[:, :len(ii)] = x[b][ii].T
    cxT = np.zeros((DIM, npj), NPBF)
    cxT[:, 1:1 + len(jj)] = context[b][jj].T

    # validity of each j row (incl. null at 0), replicated per head
    valid = (np.arange(npj) < 1 + len(jj)).astype(np.float32)
    vones = np.repeat(valid.reshape(jtc, 128).T[:, :, None], HG, axis=2)

    gs = slice(g * DG, (g + 1) * DG)
    return {
        "xT": xT,
        "cxT": cxT,
        "wq": Wq[:, gs].astype(NPBF),
        "wk": Wkv[:, gs].astype(NPBF),
        "wv": Wkv[:, DIM + g * DG: DIM + (g + 1) * DG].astype(NPBF),
        "wo": Wo[gs, :].astype(NPBF),
        "vones": np.ascontiguousarray(vones.reshape(128, jtc * HG)).astype(NPBF),
        "nk": np.ascontiguousarray(np.tile(null_key, 2).reshape(128, 1)),
        "nv": np.tile(null_value, HG).reshape(1, DG).astype(NPBF),
    }


def kernel(x, context, mask, context_mask, Wq, Wkv, Wo, bo, null_key, null_value):
    global LAST_RESULTS, LAST_NC
    inputs = {
        "x": x, "context": context, "mask": mask, "context_mask": context_mask,
        "Wq": Wq, "Wkv": Wkv, "Wo": Wo, "bo": bo,
        "null_key": null_key, "null_value": null_value,
    }
    mask_np = np.asarray(mask, bool)
    cm_np = np.asarray(context_mask, bool)
    idx_i = [np.nonzero(mask_np[b])[0] for b in range(B)]
    idx_j = [np.nonzero(cm_np[b])[0] for b in range(B)]
    npi = max(128, -(-max(len(ii) for ii in idx_i) // 128) * 128)
    npj = max(128, -(-max(1 + len(jj) for jj in idx_j) // 128) * 128)

    key = (npi, npj)
    if key not in _CACHE:
        _CACHE[key] = _build(npi, npj)
    nc = _CACHE[key]
    LAST_NC = nc

    in_maps = [_core_inputs(inputs, core, npi, npj, idx_i, idx_j)
               for core in range(8)]
    res = bass_utils.run_bass_kernel_spmd(nc, in_maps, core_ids=list(range(8)))
    LAST_RESULTS = res

    Wkv_np = np.asarray(Wkv, np.float32)
    Wo_np = np.asarray(Wo, np.float32)
    bo_np = np.asarray(bo, np.float32)
    nv_full = np.tile(np.asarray(null_value, np.float32), HEADS)

    out = np.empty((B, N, DIM), np.float32)
    for b in range(B):
        nact = len(idx_i[b])
        if nact:
            s = (res.results[2 * b]["out"][:nact]
                 + res.results[2 * b + 1]["out"][:nact] + bo_np)
            out[b][idx_i[b]] = s
        # masked queries attend uniformly over ALL m+1 positions
        vsum = np.asarray(context[b], np.float32).sum(0) @ Wkv_np[:, INNER:]
        urow = (vsum + nv_full) / (M + 1) @ Wo_np + bo_np
        out[b][~mask_np[b]] = urow
    return out


# revision 29
# speedup vs baseline: 7.6841x; 1.0170x over previous
"""Cross-attention kernel for Trainium2, distributed over 8 NeuronCores.

Sharding: data-parallel over batch (4) x tensor-parallel over head groups (2).
Core c handles batch b = c//2, heads [4g, 4g+4) with g = c%2.

Key structural ideas (vs. a dense implementation):

* Host-side compaction. Masked queries (mask[b,i]=False) all produce the
  SAME output row: softmax over an all-masked row is uniform over all m+1
  positions, so out_i = (sum_j v_j + nv)/(m+1) @ Wo + bo — computed on the
  host. Masked context positions contribute exactly 0 after softmax. The
  device only sees the ~50% active queries and ~50% unmasked context
  columns (null token at column 0), cutting attention work ~4x. Padding
  to 128 multiples: pad queries are zero columns (output discarded); pad
  context columns are zeroed and excluded from softmax by a zero in the
  ones-column of the augmented v (so they add 0 to both numerator and
  denominator — no mask bias needed anywhere).

* bf16 matmul operands everywhere; PSUM accumulation stays fp32. PE runs
  1 cycle/row for bf16 vs 4 for fp32. Tolerance is 2e-2; bf16 lands ~6e-3.

* The attention inner loop is Act-engine-paced (exp is Act-only). Per
  (i-chunk, head-pair): S matmuls and exp run 2 j-tiles ahead of the
  attn@v matmuls (PSUM: 3 score bufs x 2 banks + 1 accumulator x 2 banks),
  so PE never blocks on the S->exp->av latency chain. PE slack inside the
  Act-paced loop is filled with the next i-chunk's q projection and the
  previous i-chunk's output projection.

* A burst of dummy PE matmuls at t=0 keeps the tensor engine busy while
  input DMAs stream, so the p-state ramp (full clock after 3us of
  continuous execution) completes before real work starts.

* Softmax denominator: v is augmented with a ones column (row 64 of each
  head's accumulation). 1/den row -> bf16, broadcast across partitions by
  a K=1 matmul into PSUM, one DVE multiply per head pair.
"""

import numpy as np
import ml_dtypes

import concourse.bass as bass
import concourse.tile as tile
from concourse import bacc, bass_utils, mybir

FP = mybir.dt.float32
BF = mybir.dt.bfloat16
AF = mybir.ActivationFunctionType
NPBF = ml_dtypes.bfloat16

B, N, M, DIM = 4, 2048, 2048, 512
HEADS, DH = 8, 64
INNER = HEADS * DH
G = 2          # head groups (tensor-parallel degree)
HG = 4         # heads per group
DG = HG * DH   # 256 dims per group
SCALE = 1.0 / np.sqrt(DH)  # 0.125
VW = DH + 1    # v columns per head incl. ones column (den row)

LAST_RESULTS = None
LAST_NC = None
_CACHE = {}


_SPLIT_SKIP = (
    "InstDrain", "InstUnconditionalBranch", "InstCall",
    "InstEventSemaphore", "InstRegisterMove", "InstDmaTrigger",
)


def _split_multi_waits(nc):
    """TRN2 TPB instruction structs accept only ONE sync wait in walrus
    codegen; extra waits assigned by the Tile scheduler are silently dropped
    from the NEFF, which races on hardware. Hoist all-but-one wait onto
    standalone same-engine InstEventSemaphore instructions (sequencer-only
    waits, the same mechanism the framework itself uses) placed immediately
    before the offending instruction."""
    valid = set(mybir.EngineType) - {mybir.EngineType.Unassigned}
    total = 0
    for bb in nc.m.functions[0].blocks:
        new_insts = []
        for ins in bb.instructions:
            si = ins.sync_info
            if (
                getattr(ins, "engine", None) in valid
                and type(ins).__name__ not in _SPLIT_SKIP
                and si is not None
                and si.on_wait
                and len(si.on_wait) > 1
            ):
                waits = list(si.on_wait)
                for w in waits[:-1]:
                    total += 1
                    ev = mybir.InstEventSemaphore(
                        name=f"evsplit{total}_{ins.name}", ins=[], outs=[])
                    ev.engine = ins.engine
                    ev.sync_info = mybir.SyncInfo(on_wait=[w], on_update=[])
                    nc.inst_map[ev.name] = ev
                    new_insts.append(ev)
                si.on_wait = waits[-1:]
            new_insts.append(ins)
        bb.instructions = new_insts
    return total


def _chunks(total):
    """Split total (a multiple of 128) into <=512-sized 128-multiples,
    descending, each >=256 where possible (256 is the fp-fast-path floor
    for PE moving dims; a smaller final chunk also shrinks the kernel's
    serial tail)."""
    out, off, rem = [], 0, total
    while rem:
        take = min(rem, 512)
        if rem - take == 128:
            take = 384
        out.append((off, take))
        off += take
        rem -= take
    return out


def _build(npi, npj):
    nc = bacc.Bacc("TRN2", debug=False, num_devices=8, enable_partition_id=False)
    d = {}

    def inp(name, shape, dt):
        d[name] = nc.dram_tensor(name, shape, dt, kind="ExternalInput").ap()

    jtc = npj // 128
    inp("xT", [DIM, npi], BF)
    inp("cxT", [DIM, npj], BF)
    inp("wq", [DIM, DG], BF)
    inp("wk", [DIM, DG], BF)
    inp("wv", [DIM, DG], BF)
    inp("wo", [DG, DIM], BF)
    inp("vones", [128, jtc * HG], BF)  # 1 for valid j rows (incl null), 0 pads
    inp("nk", [128, 1], FP)            # null_key tiled x2
    inp("nv", [1, DG], BF)             # null_value tiled x4
    d["out"] = nc.dram_tensor("out", [npi, DIM], FP, kind="ExternalOutput").ap()

    with tile.TileContext(nc) as tc:
        _body(tc, d, npi, npj)
    _split_multi_waits(nc)
    nc.compile()
    return nc


def _body(tc, d, npi, npj):
    nc = tc.nc
    jtc = npj // 128
    ichunks = _chunks(npi)
    jchunks = _chunks(npj)

    with (
        tc.tile_pool(name="consts", bufs=1) as consts,
        tc.tile_pool(name="big", bufs=1) as big,
        tc.tile_pool(name="spool", bufs=4) as spool,
        tc.tile_pool(name="fop", bufs=2) as fop,
        tc.tile_pool(name="dpool", bufs=2) as dpool,
        tc.tile_pool(name="sp", bufs=3, space="PSUM") as sp_ps,
        tc.tile_pool(name="acc", bufs=1, space="PSUM") as acc_ps,
    ):
        # ---- inputs; ordered so the k projection (first consumer after
        # warmup) unblocks earliest
        wk = consts.tile([128, 4, DG], BF)
        nc.sync.dma_start(wk[:], d["wk"].rearrange("(c p) d -> p c d", p=128))
        cxTc = big.tile([128, 4, npj], BF)
        nc.sync.dma_start(cxTc[:], d["cxT"].rearrange("(c p) j -> p c j", p=128))
        wq = consts.tile([128, 4, DG], BF)
        nc.sync.dma_start(wq[:], d["wq"].rearrange("(c p) d -> p c d", p=128))
        xTc = big.tile([128, 4, npi], BF)
        nc.sync.dma_start(xTc[:], d["xT"].rearrange("(c p) i -> p c i", p=128))
        wv = consts.tile([128, 4, DG], BF)
        nc.sync.dma_start(wv[:], d["wv"].rearrange("(c p) d -> p c d", p=128))
        wo = consts.tile([128, 2, DIM], BF)
        nc.sync.dma_start(wo[:], d["wo"].rearrange("(c p) o -> p c o", p=128))
        nk = consts.tile([128, 1], FP)
        nc.sync.dma_start(nk[:], d["nk"])

        qT = big.tile([128, 2, npi], BF)
        kT = big.tile([128, 2, npj], BF)
        vsb = big.tile([128, jtc, HG, VW], BF)
        Osb = big.tile([128, 2, npi], BF)
        nc.sync.dma_start(vsb[:, :, :, DH:VW],
                          d["vones"].rearrange("p (j h) -> p j h", h=HG))

        ones_row = consts.tile([1, DH], BF)
        nc.vector.memset(ones_row[:], 1.0)

        # ---- PE warmup: dummy matmuls keep the p-state ramp going while
        # the input DMAs stream (ramp hits full clock after 3us busy)
        wsrc = consts.tile([128, 256], BF)
        nc.vector.memset(wsrc[:], 0.5)
        wps = sp_ps.tile([128, 2, 512], FP, tag="sp", name="warm")
        for i in range(6):
            nc.tensor.matmul(wps[:, i % 2, 0:256], wsrc[:, 0:128], wsrc[:],
                             start=True, stop=True)

        def qproj(ci):
            off, cs = ichunks[ci]
            ps = sp_ps.tile([128, 2, 512], FP, tag="sp", name=f"psq{off}")
            for dc in range(2):
                for cc in range(4):
                    nc.tensor.matmul(
                        ps[:, dc, :cs],
                        wq[:, cc, dc * 128:(dc + 1) * 128],
                        xTc[:, cc, off:off + cs],
                        start=(cc == 0), stop=(cc == 3),
                    )
            nc.scalar.activation(qT[:, :, off:off + cs], ps[:, :, :cs], AF.Tanh)

        def outproj(ci):
            off, cs = ichunks[ci]
            for t in range(cs // 128):
                it = off // 128 + t
                pf = sp_ps.tile([128, 2, 512], FP, tag="sp", name=f"pf{it}")
                for dc in range(2):
                    nc.tensor.matmul(
                        pf[:, 0, :],
                        Osb[:, dc, it * 128:(it + 1) * 128],
                        wo[:, dc, :],
                        start=(dc == 0), stop=(dc == 1),
                    )
                fo = fop.tile([128, 512], FP, tag="fo", name=f"fo{it}")
                nc.vector.tensor_copy(fo[:], pf[:, 0, :])
                nc.sync.dma_start(d["out"][it * 128:(it + 1) * 128, :], fo[:])

        # ---- projections needed before attention: all k, q chunk 0, all v
        for off, cs in jchunks:
            ps = sp_ps.tile([128, 2, 512], FP, tag="sp", name=f"psk{off}")
            for dc in range(2):
                for cc in range(4):
                    nc.tensor.matmul(
                        ps[:, dc, :cs],
                        wk[:, cc, dc * 128:(dc + 1) * 128],
                        cxTc[:, cc, off:off + cs],
                        start=(cc == 0), stop=(cc == 3),
                    )
            nc.scalar.activation(kT[:, :, off:off + cs], ps[:, :, :cs], AF.Tanh)
        for dc in range(2):
            nc.scalar.activation(kT[:, dc, 0:1], nk[:], AF.Tanh)

        qproj(0)

        for jt0 in range(0, jtc, 2):
            ps = sp_ps.tile([128, 2, 512], FP, tag="sp", name=f"psv{jt0}")
            for s in range(2):
                jt = jt0 + s
                if jt >= jtc:
                    break
                for cc in range(4):
                    nc.tensor.matmul(
                        ps[:, s, 0:DG],
                        cxTc[:, cc, jt * 128:(jt + 1) * 128],
                        wv[:, cc, :],
                        start=(cc == 0), stop=(cc == 3),
                    )
                nc.vector.tensor_copy(
                    vsb[:, jt, :, 0:DH],
                    ps[:, s, 0:DG].rearrange("p (h e) -> p h e", h=HG),
                )
        # null token value at j=0 — must land after the vproj copy of tile 0
        nc.sync.dma_start(vsb[0:1, 0, :, 0:DH],
                          d["nv"].rearrange("a (h e) -> a h e", h=HG))

        # ---- attention: one pipelined stream over (i-chunk, head-pair)
        # segments x j tiles. S matmuls + exp run 2 j-tiles ahead of the
        # attn@v matmuls, ACROSS segment boundaries, so the PE never sits
        # through the Act engine's exp drain at a segment's tail. Each
        # segment's denominator division is emitted right after its last
        # attn@v (i.e. inside the next segment's stream); PE slack inside
        # the Act-paced loop is filled with q/out projections.
        nic = len(ichunks)
        segs = [(ci, hp) for ci in range(nic) for hp in range(2)]
        po_of = {}

        def emit_av(item):
            ssb, jt, ci, hp = item
            off, cs = ichunks[ci]
            if jt == 0:  # lazily created so pool-buffer order == use order
                po_of[(ci, hp)] = acc_ps.tile([128, 2, 512], FP, tag="po",
                                              name=f"po{ci}{hp}")
            po2 = po_of[(ci, hp)]
            for hh in range(2):
                nc.tensor.matmul(
                    po2[0:VW, hh, :cs],
                    vsb[:, jt, 2 * hp + hh, :],
                    ssb[:, hh, :cs],
                    start=(jt == 0), stop=(jt == jtc - 1),
                )
            if jt == jtc - 1:
                den_div(ci, hp)

        def den_div(ci, hp):
            # divide by denominator (row DH of each head's po2)
            off, cs = ichunks[ci]
            po2 = po_of[(ci, hp)]
            posb = dpool.tile([128, 2, 512], FP, tag="posb")
            nc.vector.tensor_copy(posb[0:VW, :, :cs], po2[0:VW, :, :cs])
            den_r = dpool.tile([1, 2, 512], BF, tag="den")
            with nc.allow_low_precision(reason="bf16 1/den; tol 2e-2"):
                nc.vector.reciprocal(den_r[:, :, :cs], posb[DH:VW, :, :cs])
            pr = sp_ps.tile([128, 2, 512], FP, tag="sp", name=f"pr{ci}{hp}")
            for s in range(2):
                nc.tensor.matmul(pr[0:DH, s, :cs], ones_row[:],
                                 den_r[0:1, s, :cs], start=True, stop=True)
            tmpo = dpool.tile([64, 2, 512], BF, tag="tmpo")
            nc.vector.tensor_mul(tmpo[:, :, :cs], posb[0:DH, :, :cs],
                                 pr[0:DH, :, :cs])
            for s in range(2):
                nc.sync.dma_start(
                    Osb[64 * s:64 * s + DH, hp, off:off + cs],
                    tmpo[:, s, :cs])

        pend = []
        for ci, hp in segs:
            off, cs = ichunks[ci]
            for jt in range(jtc):
                sps = sp_ps.tile([128, 2, 512], FP, tag="sp",
                                 name=f"s{ci}_{hp}_{jt}")
                for hh in range(2):
                    nc.tensor.matmul(
                        sps[:, hh, :cs],
                        kT[64 * hh:64 * hh + DH, hp, jt * 128:(jt + 1) * 128],
                        qT[64 * hh:64 * hh + DH, hp, off:off + cs],
                        start=True, stop=True,
                    )
                ssb = spool.tile([128, 2, 512], BF, tag="s",
                                 name=f"e{ci}_{hp}_{jt}")
                nc.scalar.activation(ssb[:, :, :cs], sps[:, :, :cs],
                                     AF.Exp, scale=float(SCALE))
                pend.append((ssb, jt, ci, hp))
                if len(pend) > 2:
                    emit_av(pend.pop(0))
            # PE filler between segments while Act drains pending exps
            if hp == 0 and ci + 1 < nic:
                qproj(ci + 1)
            if hp == 1 and ci > 0:
                outproj(ci - 1)
        while pend:
            emit_av(pend.pop(0))
        outproj(nic - 1)


def _core_inputs(inputs, core, npi, npj, idx_i, idx_j):
    b, g = core // 2, core % 2
    x = np.asarray(inputs["x"], np.float32)
    context = np.asarray(inputs["context"], np.float32)
    Wq = np.asarray(inputs["Wq"], np.float32)
    Wkv = np.asarray(inputs["Wkv"], np.float32)
    Wo = np.asarray(inputs["Wo"], np.float32)
    null_key = np.asarray(inputs["null_key"], np.float32)
    null_value = np.asarray(inputs["null_value"], np.float32)

    ii, jj = idx_i[b], idx_j[b]
    jtc = npj // 128

    xT = np.zeros((DIM, npi), NPBF)
    xT[:, :len(ii)] = x[b][ii].T
    cxT = np.zeros((DIM, npj), NPBF)
    cxT[:, 1:1 + len(jj)] = context[b][jj].T

    # validity of each j row (incl. null at 0), replicated per head
    valid = (np.arange(npj) < 1 + len(jj)).astype(np.float32)
    vones = np.repeat(valid.reshape(jtc, 128).T[:, :, None], HG, axis=2)

    gs = slice(g * DG, (g + 1) * DG)
    return {
        "xT": xT,
        "cxT": cxT,
        "wq": Wq[:, gs].astype(NPBF),
        "wk": Wkv[:, gs].astype(NPBF),
        "wv": Wkv[:, DIM + g * DG: DIM + (g + 1) * DG].astype(NPBF),
        "wo": Wo[gs, :].astype(NPBF),
        "vones": np.ascontiguousarray(vones.reshape(128, jtc * HG)).astype(NPBF),
        "nk": np.ascontiguousarray(np.tile(null_key, 2).reshape(128, 1)),
        "nv": np.tile(null_value, HG).reshape(1, DG).astype(NPBF),
    }


def kernel(x, context, mask, context_mask, Wq, Wkv, Wo, bo, null_key, null_value):
    global LAST_RESULTS, LAST_NC
    inputs = {
        "x": x, "context": context, "mask": mask, "context_mask": context_mask,
        "Wq": Wq, "Wkv": Wkv, "Wo": Wo, "bo": bo,
        "null_key": null_key, "null_value": null_value,
    }
    mask_np = np.asarray(mask, bool)
    cm_np = np.asarray(context_mask, bool)
    idx_i = [np.nonzero(mask_np[b])[0] for b in range(B)]
    idx_j = [np.nonzero(cm_np[b])[0] for b in range(B)]
    npi = max(128, -(-max(len(ii) for ii in idx_i) // 128) * 128)
    npj = max(128, -(-max(1 + len(jj) for jj in idx_j) // 128) * 128)

    key = (npi, npj)
    if key not in _CACHE:
        _CACHE[key] = _build(npi, npj)
    nc = _CACHE[key]
    LAST_NC = nc

    in_maps = [_core_inputs(inputs, core, npi, npj, idx_i, idx_j)
               for core in range(8)]
    res = bass_utils.run_bass_kernel_spmd(nc, in_maps, core_ids=list(range(8)))
    LAST_RESULTS = res

    Wkv_np = np.asarray(Wkv, np.float32)
    Wo_np = np.asarray(Wo, np.float32)
    bo_np = np.asarray(bo, np.float32)
    nv_full = np.tile(np.asarray(null_value, np.float32), HEADS)

    out = np.empty((B, N, DIM), np.float32)
    for b in range(B):
        nact = len(idx_i[b])
        if nact:
            s = (res.results[2 * b]["out"][:nact]
                 + res.results[2 * b + 1]["out"][:nact] + bo_np)
            out[b][idx_i[b]] = s
        # masked queries attend uniformly over ALL m+1 positions
        vsum = np.asarray(context[b], np.float32).sum(0) @ Wkv_np[:, INNER:]
        urow = (vsum + nv_full) / (M + 1) @ Wo_np + bo_np
        out[b][~mask_np[b]] = urow
    return out


# revision 36
# speedup vs baseline: 7.7295x; 1.0059x over previous
"""Cross-attention kernel for Trainium2, distributed over 8 NeuronCores.

Sharding: data-parallel over batch (4) x tensor-parallel over head groups (2).
Core c handles batch b = c//2, heads [4g, 4g+4) with g = c%2.

Key structural ideas (vs. a dense implementation):

* Host-side compaction. Masked queries (mask[b,i]=False) all produce the
  SAME output row: softmax over an all-masked row is uniform over all m+1
  positions, so out_i = (sum_j v_j + nv)/(m+1) @ Wo + bo — computed on the
  host. Masked context positions contribute exactly 0 after softmax. The
  device only sees the ~50% active queries and ~50% unmasked context
  columns (null token at column 0), cutting attention work ~4x. Padding
  to 128 multiples: pad queries are zero columns (output discarded); pad
  context columns are zeroed and excluded from softmax by a zero in the
  ones-column of the augmented v (so they add 0 to both numerator and
  denominator — no mask bias needed anywhere).

* bf16 matmul operands everywhere; PSUM accumulation stays fp32. PE runs
  1 cycle/row for bf16 vs 4 for fp32. Tolerance is 2e-2; bf16 lands ~6e-3.

* The attention inner loop is Act-engine-paced (exp is Act-only). Per
  (i-chunk, head-pair): S matmuls and exp run 2 j-tiles ahead of the
  attn@v matmuls (PSUM: 3 score bufs x 2 banks + 1 accumulator x 2 banks),
  so PE never blocks on the S->exp->av latency chain. PE slack inside the
  Act-paced loop is filled with the next i-chunk's q projection and the
  previous i-chunk's output projection.

* A burst of dummy PE matmuls at t=0 keeps the tensor engine busy while
  input DMAs stream, so the p-state ramp (full clock after 3us of
  continuous execution) completes before real work starts.

* Softmax denominator: v is augmented with a ones column (row 64 of each
  head's accumulation). 1/den row -> bf16, broadcast across partitions by
  a K=1 matmul into PSUM, one DVE multiply per head pair.
"""

import numpy as np
import ml_dtypes

import concourse.bass as bass
import concourse.tile as tile
from concourse import bacc, bass_utils, mybir

FP = mybir.dt.float32
BF = mybir.dt.bfloat16
AF = mybir.ActivationFunctionType
NPBF = ml_dtypes.bfloat16

B, N, M, DIM = 4, 2048, 2048, 512
HEADS, DH = 8, 64
INNER = HEADS * DH
G = 2          # head groups (tensor-parallel degree)
HG = 4         # heads per group
DG = HG * DH   # 256 dims per group
SCALE = 1.0 / np.sqrt(DH)  # 0.125
VW = DH + 1    # v columns per head incl. ones column (den row)

LAST_RESULTS = None
LAST_NC = None
_CACHE = {}


_SPLIT_SKIP = (
    "InstDrain", "InstUnconditionalBranch", "InstCall",
    "InstEventSemaphore", "InstRegisterMove", "InstDmaTrigger",
)


def _split_multi_waits(nc):
    """TRN2 TPB instruction structs accept only ONE sync wait in walrus
    codegen; extra waits assigned by the Tile scheduler are silently dropped
    from the NEFF, which races on hardware. Hoist all-but-one wait onto
    standalone same-engine InstEventSemaphore instructions (sequencer-only
    waits, the same mechanism the framework itself uses) placed immediately
    before the offending instruction."""
    valid = set(mybir.EngineType) - {mybir.EngineType.Unassigned}
    total = 0
    for bb in nc.m.functions[0].blocks:
        new_insts = []
        for ins in bb.instructions:
            si = ins.sync_info
            if (
                getattr(ins, "engine", None) in valid
                and type(ins).__name__ not in _SPLIT_SKIP
                and si is not None
                and si.on_wait
                and len(si.on_wait) > 1
            ):
                waits = list(si.on_wait)
                for w in waits[:-1]:
                    total += 1
                    ev = mybir.InstEventSemaphore(
                        name=f"evsplit{total}_{ins.name}", ins=[], outs=[])
                    ev.engine = ins.engine
                    ev.sync_info = mybir.SyncInfo(on_wait=[w], on_update=[])
                    nc.inst_map[ev.name] = ev
                    new_insts.append(ev)
                si.on_wait = waits[-1:]
            new_insts.append(ins)
        bb.instructions = new_insts
    return total


def _chunks(total):
    """Split total (a multiple of 128) into <=512-sized 128-multiples,
    descending, each >=256 where possible (256 is the fp-fast-path floor
    for PE moving dims; a smaller final chunk also shrinks the kernel's
    serial tail)."""
    out, off, rem = [], 0, total
    while rem:
        take = min(rem, 512)
        if rem - take == 128:
            take = 384
        out.append((off, take))
        off += take
        rem -= take
    return out


def _build(npi, npj):
    nc = bacc.Bacc("TRN2", debug=False, num_devices=8, enable_partition_id=False)
    d = {}

    def inp(name, shape, dt):
        d[name] = nc.dram_tensor(name, shape, dt, kind="ExternalInput").ap()

    jtc = npj // 128
    inp("xT", [DIM, npi], BF)
    inp("cxT", [DIM, npj], BF)
    inp("wq", [DIM, DG], BF)
    inp("wk", [DIM, DG], BF)
    inp("wv", [DIM, DG], BF)
    inp("wo", [DG, DIM], BF)
    inp("vones", [128, jtc * HG], BF)  # 1 for valid j rows (incl null), 0 pads
    inp("nk", [128, 1], FP)            # null_key tiled x2
    inp("nv", [1, DG], BF)             # null_value tiled x4
    d["out"] = nc.dram_tensor("out", [npi, DIM], FP, kind="ExternalOutput").ap()

    with tile.TileContext(nc) as tc:
        _body(tc, d, npi, npj)
    nc.compile()
    return nc


def _body(tc, d, npi, npj):
    nc = tc.nc
    jtc = npj // 128
    ichunks = _chunks(npi)
    jchunks = _chunks(npj)

    with (
        tc.tile_pool(name="consts", bufs=1) as consts,
        tc.tile_pool(name="big", bufs=1) as big,
        tc.tile_pool(name="spool", bufs=4) as spool,
        tc.tile_pool(name="fop", bufs=2) as fop,
        tc.tile_pool(name="dpool", bufs=2) as dpool,
        tc.tile_pool(name="sp", bufs=3, space="PSUM") as sp_ps,
        tc.tile_pool(name="acc", bufs=1, space="PSUM") as acc_ps,
    ):
        # ---- inputs. One whole tile per DMA (sliced DMA writes into a
        # shared tile mis-sync at the NEFF level — see module docstring);
        # x/ctx are split into per-chunk tiles so each projection chunk can
        # start as soon as its own transfer lands. Ordered so the k
        # projection (first consumer after warmup) unblocks earliest.
        wk = consts.tile([128, 4, DG], BF)
        nc.sync.dma_start(wk[:], d["wk"].rearrange("(c p) d -> p c d", p=128))
        cxSrc = d["cxT"].rearrange("(c p) j -> p c j", p=128)
        xSrc = d["xT"].rearrange("(c p) i -> p c i", p=128)
        cxTt, xTt = [], []
        cxTt.append(big.tile([128, 4, jchunks[0][1]], BF, name="cxT0"))
        nc.sync.dma_start(cxTt[0][:], cxSrc[:, :, 0:jchunks[0][1]])
        wq = consts.tile([128, 4, DG], BF)
        nc.sync.dma_start(wq[:], d["wq"].rearrange("(c p) d -> p c d", p=128))
        xTt.append(big.tile([128, 4, ichunks[0][1]], BF, name="xT0"))
        nc.sync.dma_start(xTt[0][:], xSrc[:, :, 0:ichunks[0][1]])
        for c, (off, cs) in enumerate(jchunks[1:], 1):
            t = big.tile([128, 4, cs], BF, name=f"cxT{c}")
            nc.sync.dma_start(t[:], cxSrc[:, :, off:off + cs])
            cxTt.append(t)
        for c, (off, cs) in enumerate(ichunks[1:], 1):
            t = big.tile([128, 4, cs], BF, name=f"xT{c}")
            nc.sync.dma_start(t[:], xSrc[:, :, off:off + cs])
            xTt.append(t)
        wv = consts.tile([128, 4, DG], BF)
        nc.sync.dma_start(wv[:], d["wv"].rearrange("(c p) d -> p c d", p=128))
        wo = consts.tile([128, 2, DIM], BF)
        nc.sync.dma_start(wo[:], d["wo"].rearrange("(c p) o -> p c o", p=128))
        nk = consts.tile([128, 1], FP)
        nc.sync.dma_start(nk[:], d["nk"])

        def cx_loc(j0):
            """Map a global j column offset to (chunk tile, local offset)."""
            for c, (off, cs) in enumerate(jchunks):
                if j0 < off + cs:
                    return cxTt[c], j0 - off
            raise AssertionError(j0)

        qT = big.tile([128, 2, npi], BF)
        kT = big.tile([128, 2, npj], BF)
        vsb = big.tile([128, jtc, HG, VW], BF)
        Osb = big.tile([128, 2, npi], BF)
        # vones/nv bounce through whole tiles + engine copies: sliced DMA
        # writes into vsb are not reliably ordered against its readers
        vot = consts.tile([128, jtc * HG], BF)
        nc.sync.dma_start(vot[:], d["vones"])
        nc.vector.tensor_copy(
            vsb[:, :, :, DH:VW],
            vot[:].rearrange("p (j h o) -> p j h o", h=HG, o=1))
        nvt = consts.tile([1, DG], BF)
        nc.sync.dma_start(nvt[:], d["nv"])

        ones_row = consts.tile([1, DH], BF)
        nc.vector.memset(ones_row[:], 1.0)

        # ---- PE warmup: dummy matmuls keep the p-state ramp going while
        # the input DMAs stream (ramp hits full clock after 3us busy)
        wsrc = consts.tile([128, 256], BF)
        nc.vector.memset(wsrc[:], 0.5)
        wps = sp_ps.tile([128, 2, 512], FP, tag="sp", name="warm")
        for i in range(6):
            nc.tensor.matmul(wps[:, i % 2, 0:256], wsrc[:, 0:128], wsrc[:],
                             start=True, stop=True)

        def qproj(ci):
            off, cs = ichunks[ci]
            ps = sp_ps.tile([128, 2, 512], FP, tag="sp", name=f"psq{off}")
            for dc in range(2):
                for cc in range(4):
                    nc.tensor.matmul(
                        ps[:, dc, :cs],
                        wq[:, cc, dc * 128:(dc + 1) * 128],
                        xTt[ci][:, cc, :cs],
                        start=(cc == 0), stop=(cc == 3),
                    )
            nc.scalar.activation(qT[:, :, off:off + cs], ps[:, :, :cs], AF.Tanh)

        def outproj(ci):
            off, cs = ichunks[ci]
            for t in range(cs // 128):
                it = off // 128 + t
                pf = sp_ps.tile([128, 2, 512], FP, tag="sp", name=f"pf{it}")
                for dc in range(2):
                    nc.tensor.matmul(
                        pf[:, 0, :],
                        Osb[:, dc, it * 128:(it + 1) * 128],
                        wo[:, dc, :],
                        start=(dc == 0), stop=(dc == 1),
                    )
                fo = fop.tile([128, 512], FP, tag="fo", name=f"fo{it}")
                nc.vector.tensor_copy(fo[:], pf[:, 0, :])
                nc.sync.dma_start(d["out"][it * 128:(it + 1) * 128, :], fo[:])

        # ---- projections needed before attention: all k, q chunk 0, all v
        for c, (off, cs) in enumerate(jchunks):
            ps = sp_ps.tile([128, 2, 512], FP, tag="sp", name=f"psk{off}")
            for dc in range(2):
                for cc in range(4):
                    nc.tensor.matmul(
                        ps[:, dc, :cs],
                        wk[:, cc, dc * 128:(dc + 1) * 128],
                        cxTt[c][:, cc, :cs],
                        start=(cc == 0), stop=(cc == 3),
                    )
            nc.scalar.activation(kT[:, :, off:off + cs], ps[:, :, :cs], AF.Tanh)
        for dc in range(2):
            nc.scalar.activation(kT[:, dc, 0:1], nk[:], AF.Tanh)

        qproj(0)

        for jt0 in range(0, jtc, 2):
            ps = sp_ps.tile([128, 2, 512], FP, tag="sp", name=f"psv{jt0}")
            for s in range(2):
                jt = jt0 + s
                if jt >= jtc:
                    break
                src, loc = cx_loc(jt * 128)
                for cc in range(4):
                    nc.tensor.matmul(
                        ps[:, s, 0:DG],
                        src[:, cc, loc:loc + 128],
                        wv[:, cc, :],
                        start=(cc == 0), stop=(cc == 3),
                    )
                nc.vector.tensor_copy(
                    vsb[:, jt, :, 0:DH],
                    ps[:, s, 0:DG].rearrange("p (h e) -> p h e", h=HG),
                )
        # null token value at j=0 — must land after the vproj copy of tile 0
        nc.vector.tensor_copy(vsb[0:1, 0, :, 0:DH],
                              nvt[:].rearrange("a (h e) -> a h e", h=HG))

        # ---- attention: one pipelined stream over (i-chunk, head-pair)
        # segments x j tiles. S matmuls + exp run 2 j-tiles ahead of the
        # attn@v matmuls, ACROSS segment boundaries, so the PE never sits
        # through the Act engine's exp drain at a segment's tail. Each
        # segment's denominator division is emitted right after its last
        # attn@v (i.e. inside the next segment's stream); PE slack inside
        # the Act-paced loop is filled with q/out projections.
        nic = len(ichunks)
        segs = [(ci, hp) for ci in range(nic) for hp in range(2)]
        po_of = {}

        def emit_av(item):
            ssb, jt, ci, hp = item
            off, cs = ichunks[ci]
            if jt == 0:  # lazily created so pool-buffer order == use order
                po_of[(ci, hp)] = acc_ps.tile([128, 2, 512], FP, tag="po",
                                              name=f"po{ci}{hp}")
            po2 = po_of[(ci, hp)]
            for hh in range(2):
                nc.tensor.matmul(
                    po2[0:VW, hh, :cs],
                    vsb[:, jt, 2 * hp + hh, :],
                    ssb[:, hh, :cs],
                    start=(jt == 0), stop=(jt == jtc - 1),
                )
            if jt == jtc - 1:
                den_div(ci, hp)

        def den_div(ci, hp):
            # divide by denominator (row DH of each head's po2)
            off, cs = ichunks[ci]
            po2 = po_of[(ci, hp)]
            posb = dpool.tile([128, 2, 512], FP, tag="posb")
            nc.vector.tensor_copy(posb[0:VW, :, :cs], po2[0:VW, :, :cs])
            den_r = dpool.tile([1, 2, 512], BF, tag="den")
            with nc.allow_low_precision(reason="bf16 1/den; tol 2e-2"):
                nc.vector.reciprocal(den_r[:, :, :cs], posb[DH:VW, :, :cs])
            pr = sp_ps.tile([128, 2, 512], FP, tag="sp", name=f"pr{ci}{hp}")
            for s in range(2):
                nc.tensor.matmul(pr[0:DH, s, :cs], ones_row[:],
                                 den_r[0:1, s, :cs], start=True, stop=True)
            tmpo = dpool.tile([64, 2, 512], BF, tag="tmpo")
            nc.vector.tensor_mul(tmpo[:, :, :cs], posb[0:DH, :, :cs],
                                 pr[0:DH, :, :cs])
            for s in range(2):
                nc.sync.dma_start(
                    Osb[64 * s:64 * s + DH, hp, off:off + cs],
                    tmpo[:, s, :cs])

        pend = []
        for ci, hp in segs:
            off, cs = ichunks[ci]
            for jt in range(jtc):
                sps = sp_ps.tile([128, 2, 512], FP, tag="sp",
                                 name=f"s{ci}_{hp}_{jt}")
                for hh in range(2):
                    nc.tensor.matmul(
                        sps[:, hh, :cs],
                        kT[64 * hh:64 * hh + DH, hp, jt * 128:(jt + 1) * 128],
                        qT[64 * hh:64 * hh + DH, hp, off:off + cs],
                        start=True, stop=True,
                    )
                ssb = spool.tile([128, 2, 512], BF, tag="s",
                                 name=f"e{ci}_{hp}_{jt}")
                nc.scalar.activation(ssb[:, :, :cs], sps[:, :, :cs],
                                     AF.Exp, scale=float(SCALE))
                pend.append((ssb, jt, ci, hp))
                if len(pend) > 2:
                    emit_av(pend.pop(0))
            # PE filler between segments while Act drains pending exps
            if hp == 0 and ci + 1 < nic:
                qproj(ci + 1)
            if hp == 1 and ci > 0:
                outproj(ci - 1)
        while pend:
            emit_av(pend.pop(0))
        outproj(nic - 1)


def _core_inputs(inputs, core, npi, npj, idx_i, idx_j):
    b, g = core // 2, core % 2
    x = np.asarray(inputs["x"], np.float32)
    context = np.asarray(inputs["context"], np.float32)
    Wq = np.asarray(inputs["Wq"], np.float32)
    Wkv = np.asarray(inputs["Wkv"], np.float32)
    Wo = np.asarray(inputs["Wo"], np.float32)
    null_key = np.asarray(inputs["null_key"], np.float32)
    null_value = np.asarray(inputs["null_value"], np.float32)

    ii, jj = idx_i[b], idx_j[b]
    jtc = npj // 128

    xT = np.zeros((DIM, npi), NPBF)
    xT[:, :len(ii)] = x[b][ii].T
    cxT = np.zeros((DIM, npj), NPBF)
    cxT[:, 1:1 + len(jj)] = context[b][jj].T

    # validity of each j row (incl. null at 0), replicated per head
    valid = (np.arange(npj) < 1 + len(jj)).astype(np.float32)
    vones = np.repeat(valid.reshape(jtc, 128).T[:, :, None], HG, axis=2)

    gs = slice(g * DG, (g + 1) * DG)
    return {
        "xT": xT,
        "cxT": cxT,
        "wq": Wq[:, gs].astype(NPBF),
        "wk": Wkv[:, gs].astype(NPBF),
        "wv": Wkv[:, DIM + g * DG: DIM + (g + 1) * DG].astype(NPBF),
        "wo": Wo[gs, :].astype(NPBF),
        "vones": np.ascontiguousarray(vones.reshape(128, jtc * HG)).astype(NPBF),
        "nk": np.ascontiguousarray(np.tile(null_key, 2).reshape(128, 1)),
        "nv": np.tile(null_value, HG).reshape(1, DG).astype(NPBF),
    }


def kernel(x, context, mask, context_mask, Wq, Wkv, Wo, bo, null_key, null_value):
    global LAST_RESULTS, LAST_NC
    inputs = {
        "x": x, "context": context, "mask": mask, "context_mask": context_mask,
        "Wq": Wq, "Wkv": Wkv, "Wo": Wo, "bo": bo,
        "null_key": null_key, "null_value": null_value,
    }
    mask_np = np.asarray(mask, bool)
    cm_np = np.asarray(context_mask, bool)
    idx_i = [np.nonzero(mask_np[b])[0] for b in range(B)]
    idx_j = [np.nonzero(cm_np[b])[0] for b in range(B)]
    npi = max(128, -(-max(len(ii) for ii in idx_i) // 128) * 128)
    npj = max(128, -(-max(1 + len(jj) for jj in idx_j) // 128) * 128)

    key = (npi, npj)
    if key not in _CACHE:
        _CACHE[key] = _build(npi, npj)
    nc = _CACHE[key]
    LAST_NC = nc

    in_maps = [_core_inputs(inputs, core, npi, npj, idx_i, idx_j)
               for core in range(8)]
    res = bass_utils.run_bass_kernel_spmd(nc, in_maps, core_ids=list(range(8)))
    LAST_RESULTS = res

    Wkv_np = np.asarray(Wkv, np.float32)
    Wo_np = np.asarray(Wo, np.float32)
    bo_np = np.asarray(bo, np.float32)
    nv_full = np.tile(np.asarray(null_value, np.float32), HEADS)

    out = np.empty((B, N, DIM), np.float32)
    for b in range(B):
        nact = len(idx_i[b])
        if nact:
            s = (res.results[2 * b]["out"][:nact]
                 + res.results[2 * b + 1]["out"][:nact] + bo_np)
            out[b][idx_i[b]] = s
        # masked queries attend uniformly over ALL m+1 positions
        vsum = np.asarray(context[b], np.float32).sum(0) @ Wkv_np[:, INNER:]
        urow = (vsum + nv_full) / (M + 1) @ Wo_np + bo_np
        out[b][~mask_np[b]] = urow
    return out
